# revision 26
# baseline (speedup 1.0000x reference)
"""Trainium2 Bass kernel for nn_DocREModel (doc-level relation extraction graph pooling).

Strategy (8 NeuronCores): each doc b (B=4) is split across 2 cores by attention
heads (6 heads each).  Key observation: the model only ever reads attention rows
at mention positions (<=128 distinct) and inside link spans, i.e. ~35% of the
[1024,1024] matrix.  Each core therefore device-GATHERS just those rows
(SWDGE indirect DMA with a runtime per-partition index tile), head-sums them on
the vector engine, and runs small PE matmuls:

  - att_r HBM layout [row, head*1024+col] (bf16): gathering one row pulls all 6
    heads of that row contiguously (12 KB/descriptor).
  - gather slot j<128 == mention j (EM=128 exactly), so the "onehot" gather
    matrix is the identity: mention rows of S come from a PE transpose of
    gathered chunk 0.  Span rows occupy slots >=128; a host-built slot mask
    [slots,16] reduces them to per-span row-sums uT[16,L] via PE matmul
    (transposed so each PSUM bank holds a single accumulation group).
  - mnum = S_mention^T @ [seq|1]  (context numerators + row-sums)
  - v    = (uT*maskCT)^T-transposed-back @ [seq|1], col-group-tiled 4x on the
    PE (16-row outputs packed at partition offsets 0/32/64/96; host sums).
  - memb = seq rows at mentions via a second (f32) indirect gather - no compute.
The host applies the tiny normalizations (head-count / span-length / row-sum
divides, entity pooling, 4-way logsumexp) while unsharding.
"""

import os
import sys

for _p in ("/opt/trn_rl_repo", "/root/.axon_site/_ro/trn_rl_repo"):
    if os.path.isdir(_p) and _p not in sys.path:
        sys.path.insert(0, _p)

import numpy as np

B, L, H, NH = 4, 1024, 768, 12
E, MPE, K = 32, 4, 16
EM = E * MPE              # 128 mentions per doc == gather chunk 0
TYPE_DIM = 20
OFFSET = 1
HPG = NH // 2             # heads per core (2 cores per doc)
CT = L // 128             # 8 column chunks
HA = H + 4                # seq | ones | 3 zero-pad -> 772 (row-sum in col 768)
NCHUNK_DEF = 3            # gather slots = 128*NCHUNK (>= 128 mentions + span rows)


def _build_nc(nchunk=NCHUNK_DEF, debug=False):
    import concourse.bass as bass
    import concourse.mybir as mybir
    import concourse.tile as tile
    from concourse import bacc

    f32 = mybir.dt.float32
    bf16 = mybir.dt.bfloat16
    fp8 = mybir.dt.float8e4  # e4m3
    i32 = mybir.dt.int32
    ts, ds = bass.ts, bass.ds

    nc = bacc.Bacc("TRN2", target_bir_lowering=False, debug=debug)

    att_r = nc.dram_tensor("att_r", [L, HPG * L], fp8, kind="ExternalInput")
    seqb = nc.dram_tensor("seqb", [L, HA], bf16, kind="ExternalInput")
    seqf = nc.dram_tensor("seqf", [L, H], f32, kind="ExternalInput")
    idx_att = nc.dram_tensor("idx_att", [128, nchunk], i32, kind="ExternalInput")
    idx_mem = nc.dram_tensor("idx_mem", [128, 1], i32, kind="ExternalInput")
    maskS = nc.dram_tensor("maskS", [128, nchunk * K], bf16, kind="ExternalInput")
    maskCT = nc.dram_tensor("maskCT", [K, L], f32, kind="ExternalInput")
    ident = nc.dram_tensor("ident", [128, 128], bf16, kind="ExternalInput")
    ident8 = nc.dram_tensor("ident8", [128, 128], fp8, kind="ExternalInput")
    out_mnum = nc.dram_tensor("out_mnum", [EM, HA], bf16, kind="ExternalOutput")
    out_v = nc.dram_tensor("out_v", [4 * K, HA], bf16, kind="ExternalOutput")
    out_memb = nc.dram_tensor("out_memb", [EM, H], f32, kind="ExternalOutput")

    with tile.TileContext(nc) as tc:
        with (
            tc.tile_pool(name="const", bufs=1) as constp,
            tc.tile_pool(name="gat", bufs=3) as gatp,
            tc.tile_pool(name="acc", bufs=1) as accp,
            tc.tile_pool(name="stage", bufs=1) as stagep,
            tc.tile_pool(name="pshold", bufs=1, space="PSUM") as pshold,
            tc.tile_pool(name="psrot", bufs=2, space="PSUM") as psrot,
        ):
            # ---- indices first, on the gpsimd queue itself: no cross-engine
            #      semaphore wait before the first gather's descriptor-gen ----
            idxa_s = constp.tile([128, nchunk], i32, tag="idxa", name="idxa")
            idxm_s = constp.tile([128, 1], i32, tag="idxm", name="idxm")
            nc.gpsimd.dma_start(out=idxa_s[:], in_=idx_att[:])
            nc.gpsimd.dma_start(out=idxm_s[:], in_=idx_mem[:])

            g5s = []
            for q in range(nchunk):
                g5 = gatp.tile([128, HPG * L], fp8, tag="g5", name="g5")
                nc.gpsimd.indirect_dma_start(
                    out=g5[:], out_offset=None, in_=att_r[:],
                    in_offset=bass.IndirectOffsetOnAxis(ap=idxa_s[:, q:q + 1], axis=0))
                g5s.append(g5)

            # ---- mention embeddings: pure f32 row-gather of seq ----
            memb_s = stagep.tile([128, H], f32, tag="memb", name="memb")
            nc.gpsimd.indirect_dma_start(
                out=memb_s[:], out_offset=None, in_=seqf[:],
                in_offset=bass.IndirectOffsetOnAxis(ap=idxm_s[:], axis=0))
            nc.sync.dma_start(out=out_memb[:], in_=memb_s[:])

            # ---- remaining consts; seqb rides the gpsimd queue BEHIND the
            #      gathers so its packets don't race the critical gather ----
            maskS_s = constp.tile([128, nchunk, K], bf16, tag="maskS", name="maskS")
            maskCT_s = constp.tile([K, L], f32, tag="maskCT", name="maskCT")
            ident_s = constp.tile([128, 128], bf16, tag="ident", name="ident")
            ident8_s = constp.tile([128, 128], fp8, tag="ident8", name="ident8")
            seq_s = constp.tile([128, CT, HA], bf16, tag="seqs", name="seqs")
            nc.sync.dma_start(out=maskS_s[:], in_=maskS[:].rearrange("p (q k) -> p q k", k=K))
            nc.scalar.dma_start(out=maskCT_s[:], in_=maskCT[:])
            nc.sync.dma_start(out=ident_s[:], in_=ident[:])
            nc.sync.dma_start(out=ident8_s[:], in_=ident8[:])
            nc.gpsimd.dma_start(out=seq_s[:], in_=seqb[:].rearrange("(c p) f -> p c f", p=128))

            # ---- head-sum on PE (identity-weight accumulation over 6 heads) ----
            gs_s = accp.tile([128, nchunk, L], bf16, tag="gs", name="gs")
            gtm_s = accp.tile([128, CT, 128], bf16, tag="gtm", name="gtm")
            wv_s = accp.tile([128, CT, K], bf16, tag="wv", name="wv")
            put0 = pshold.tile([K, 512], f32, tag="put0", name="put0")
            put1 = pshold.tile([K, 512], f32, tag="put1", name="put1")
            for q in range(nchunk):
                g5 = g5s[q]
                for half in range(2):
                    ph = psrot.tile([128, 512], f32, tag="ph", name="ph")
                    for h in range(HPG):
                        nc.tensor.matmul(ph[:], ident8_s[:],
                                         g5[:, ds(h * L + half * 512, 512)],
                                         start=(h == 0), stop=(h == HPG - 1))
                    nc.vector.tensor_copy(gs_s[:, q, ds(half * 512, 512)], ph[:])
                # span-row reduction (transposed): uT[k,c] += maskS^T @ gs_q
                nc.tensor.matmul(put0[:], maskS_s[:, q, :], gs_s[:, q, 0:512],
                                 start=(q == 0), stop=(q == nchunk - 1))
                nc.tensor.matmul(put1[:], maskS_s[:, q, :], gs_s[:, q, 512:L],
                                 start=(q == 0), stop=(q == nchunk - 1))
                if q == 0:
                    # mention rows of S: PE-transpose of gathered chunk 0
                    for ct in range(CT):
                        pt = psrot.tile([128, 128], bf16, tag="pt", name="pt")
                        nc.tensor.transpose(pt[:], gs_s[:, 0, ts(ct, 128)], ident_s[:])
                        nc.vector.tensor_copy(gtm_s[:, ct, :], pt[:])

            # ---- mnum: mention-context numerators (runs early, off gtm) ----
            pm0 = pshold.tile([EM, 512], f32, tag="pm0", name="pm0")
            pm1 = pshold.tile([EM, HA - 512], f32, tag="pm1", name="pm1")
            for ct in range(CT):
                st, sp = (ct == 0), (ct == CT - 1)
                nc.tensor.matmul(pm0[:], gtm_s[:, ct, :], seq_s[:, ct, 0:512], start=st, stop=sp)
                nc.tensor.matmul(pm1[:], gtm_s[:, ct, :], seq_s[:, ct, 512:HA], start=st, stop=sp)
            mnum_s = stagep.tile([EM, HA], bf16, tag="mnum", name="mnum")
            nc.scalar.copy(out=mnum_s[:, 0:512], in_=pm0[:])
            nc.scalar.copy(out=mnum_s[:, 512:HA], in_=pm1[:])
            nc.sync.dma_start(out=out_mnum[:], in_=mnum_s[:])

            # ---- wvT = uT * column-mask, transpose back, col-tiled v matmuls ----
            wvt_s = accp.tile([K, L], bf16, tag="wvt", name="wvt")
            nc.vector.tensor_mul(wvt_s[:, 0:512], put0[:], maskCT_s[:, 0:512])
            nc.vector.tensor_mul(wvt_s[:, 512:L], put1[:], maskCT_s[:, 512:L])
            for ct in range(CT):
                ptk = psrot.tile([128, 128], bf16, tag="pt", name="ptk")
                nc.tensor.transpose(ptk[:, 0:K], wvt_s[:, ts(ct, 128)], ident_s[0:K, 0:K])
                nc.vector.tensor_copy(wv_s[:, ct, :], ptk[:, 0:K])
            pv0 = psrot.tile([128, 512], f32, tag="ph", name="pv0")
            pv1 = psrot.tile([128, 512], f32, tag="ph", name="pv1")
            v_s = stagep.tile([128, HA], bf16, tag="v", name="v")
            for ct in range(CT):
                grp = ct // 2
                st, sp = (ct % 2 == 0), (ct % 2 == 1)
                nc.tensor.matmul(pv0[ds(32 * grp, K), :], wv_s[:, ct, :],
                                 seq_s[:, ct, 0:512], start=st, stop=sp,
                                 tile_position=(0, 32 * grp))
                nc.tensor.matmul(pv1[ds(32 * grp, K), 0:HA - 512], wv_s[:, ct, :],
                                 seq_s[:, ct, 512:HA], start=st, stop=sp,
                                 tile_position=(0, 32 * grp))
                if sp:
                    nc.scalar.copy(out=v_s[ds(32 * grp, K), 0:512],
                                   in_=pv0[ds(32 * grp, K), :])
                    nc.vector.tensor_copy(v_s[ds(32 * grp, K), 512:HA],
                                          pv1[ds(32 * grp, K), 0:HA - 512])
                    nc.scalar.dma_start(out=out_v[ts(grp, K), :],
                                        in_=v_s[ds(32 * grp, K), :])

    nc.compile()
    return nc


_NC_CACHE = {}


def _get_nc(nchunk=NCHUNK_DEF):
    if nchunk not in _NC_CACHE:
        _NC_CACHE[nchunk] = _build_nc(nchunk)
    return _NC_CACHE[nchunk]


def _per_core_inputs(sequence_output, attention, mention_pos, link_start, link_len):
    """Returns (in_maps for 8 cores, per-doc span lengths, nchunk)."""
    import ml_dtypes
    seq = np.ascontiguousarray(np.asarray(sequence_output, dtype=np.float32))
    att = np.asarray(attention)
    mpos = np.asarray(mention_pos).astype(np.int64)
    lstart = np.asarray(link_start).astype(np.int64)
    llen = np.asarray(link_len).astype(np.int64)

    docs = []
    max_slots = 0
    for b in range(B):
        pos = (mpos[b] + OFFSET).reshape(EM)
        s = lstart[b] + OFFSET
        e = lstart[b] + llen[b] + 1 + OFFSET
        row2slot = {}
        slots = list(pos)
        for j, r in enumerate(pos):
            row2slot.setdefault(int(r), j)
        for si, ei in zip(s, e):
            for r in range(int(si), int(ei)):
                if r not in row2slot:
                    row2slot[r] = len(slots)
                    slots.append(r)
        docs.append((pos, s, e, row2slot, slots))
        max_slots = max(max_slots, len(slots))
    nchunk = max(NCHUNK_DEF, -(-max_slots // 128))

    ident = np.eye(128, dtype=ml_dtypes.bfloat16)
    ident8 = np.eye(128, dtype=ml_dtypes.float8_e4m3fn)
    in_maps = []
    lengths = []
    for b in range(B):
        pos, s, e, row2slot, slots = docs[b]
        n_slots = nchunk * 128
        sl = np.zeros(n_slots, np.int32)
        sl[:len(slots)] = slots
        idx_att = np.ascontiguousarray(sl.reshape(nchunk, 128).T)
        idx_mem = np.ascontiguousarray(pos.astype(np.int32).reshape(128, 1))
        mS = np.zeros((n_slots, K), np.float32)
        mC = np.zeros((L, K), np.float32)
        for k, (si, ei) in enumerate(zip(s, e)):
            mC[int(si):int(ei), k] = 1.0
            for r in range(int(si), int(ei)):
                mS[row2slot[r], k] = 1.0
        maskS = np.ascontiguousarray(
            mS.reshape(nchunk, 128, K).transpose(1, 0, 2).reshape(128, nchunk * K)
        ).astype(ml_dtypes.bfloat16)
        maskCT = np.ascontiguousarray(mC.T)
        seqb = np.concatenate(
            [seq[b], np.ones((L, 1), np.float32), np.zeros((L, HA - H - 1), np.float32)],
            axis=1).astype(ml_dtypes.bfloat16)
        seqf = seq[b]
        lengths.append((e - s).astype(np.float32))
        for g in range(2):
            att_r = np.ascontiguousarray(
                att[b, g * HPG:(g + 1) * HPG].astype(ml_dtypes.float8_e4m3fn)
                .transpose(1, 0, 2).reshape(L, HPG * L))
            in_maps.append({
                "att_r": att_r, "seqb": seqb, "seqf": seqf,
                "idx_att": idx_att, "idx_mem": idx_mem,
                "maskS": maskS, "maskCT": maskCT, "ident": ident, "ident8": ident8,
            })
    return in_maps, lengths, nchunk


def _combine(outs, lengths, type_table):
    ttab = np.asarray(type_table, dtype=np.float32)
    type_ids = np.concatenate(
        [np.zeros(E, np.int64), np.ones(EM, np.int64), np.full(K, 2, np.int64)])
    nodes_type = ttab[type_ids]  # [E+EM+K, TYPE_DIM]

    out = np.zeros((B, E + EM + K + E + EM, H + TYPE_DIM), np.float32)
    for b in range(B):
        o0, o1 = outs[2 * b], outs[2 * b + 1]
        v4 = np.asarray(o0["out_v"], np.float32) + np.asarray(o1["out_v"], np.float32)
        v = v4.reshape(4, K, HA).sum(axis=0)
        mnum = (np.asarray(o0["out_mnum"], np.float32)
                + np.asarray(o1["out_mnum"], np.float32))
        memb = o0["out_memb"]
        length = lengths[b]

        link_rep = v[:, :H] / (NH * length[:, None])
        m_ctx = mnum[:, :H] / (mnum[:, H:H + 1] + NH * 1e-5)
        enum = mnum.reshape(E, MPE, HA).sum(axis=1)
        e_ctx = enum[:, :H] / (enum[:, H:H + 1] + NH * MPE * 1e-5)

        mg = memb.reshape(E, MPE, H)
        mmax = mg.max(axis=1)
        eemb = np.log(np.exp(mg - mmax[:, None, :]).sum(axis=1)) + mmax

        nodes_raw = np.concatenate([eemb, memb, link_rep], axis=0)      # [176,H]
        nodes = np.concatenate([nodes_raw, nodes_type], axis=1)         # [176,H+20]
        ctx = np.concatenate([e_ctx, m_ctx], axis=0)                    # [160,H]
        ctx = np.concatenate([ctx, np.zeros((E + EM, TYPE_DIM), np.float32)], axis=1)
        out[b] = np.concatenate([nodes, ctx], axis=0)
    return out


def kernel(**inputs):
    from concourse.bass_utils import run_bass_kernel_spmd

    in_maps, lengths, nchunk = _per_core_inputs(
        inputs["sequence_output"], inputs["attention"],
        inputs["mention_pos"], inputs["link_start"], inputs["link_len"])
    nc = _get_nc(nchunk)
    res = run_bass_kernel_spmd(nc, in_maps, core_ids=list(range(8)))
    return _combine(res.results, lengths, inputs["type_table"])


# revision 28
# speedup vs baseline: 1.0300x; 1.0300x over previous
"""Trainium2 Bass kernel for nn_DocREModel (doc-level relation extraction graph pooling).

Strategy (8 NeuronCores): each doc b (B=4) is split across 2 cores by attention
heads (6 heads each).  Key observation: the model only ever reads attention rows
at mention positions (<=128 distinct) and inside link spans, i.e. ~35% of the
[1024,1024] matrix.  Each core therefore device-GATHERS just those rows
(SWDGE indirect DMA with a runtime per-partition index tile), head-sums them on
the vector engine, and runs small PE matmuls:

  - att_r HBM layout [row, head*1024+col] (bf16): gathering one row pulls all 6
    heads of that row contiguously (12 KB/descriptor).
  - gather slot j<128 == mention j (EM=128 exactly), so the "onehot" gather
    matrix is the identity: mention rows of S come from a PE transpose of
    gathered chunk 0.  Span rows occupy slots >=128; a host-built slot mask
    [slots,16] reduces them to per-span row-sums uT[16,L] via PE matmul
    (transposed so each PSUM bank holds a single accumulation group).
  - mnum = S_mention^T @ [seq|1]  (context numerators + row-sums)
  - v    = (uT*maskCT)^T-transposed-back @ [seq|1], col-group-tiled 4x on the
    PE (16-row outputs packed at partition offsets 0/32/64/96; host sums).
  - memb = seq rows at mentions via a second (f32) indirect gather - no compute.
The host applies the tiny normalizations (head-count / span-length / row-sum
divides, entity pooling, 4-way logsumexp) while unsharding.
"""

import os
import sys

for _p in ("/opt/trn_rl_repo", "/root/.axon_site/_ro/trn_rl_repo"):
    if os.path.isdir(_p) and _p not in sys.path:
        sys.path.insert(0, _p)

import numpy as np

B, L, H, NH = 4, 1024, 768, 12
E, MPE, K = 32, 4, 16
EM = E * MPE              # 128 mentions per doc == gather chunk 0
TYPE_DIM = 20
OFFSET = 1
HPG = NH // 2             # heads per core (2 cores per doc)
CT = L // 128             # 8 column chunks
HA = H + 4                # seq | ones | 3 zero-pad -> 772 (row-sum in col 768)
NCHUNK_DEF = 3            # gather slots = 128*NCHUNK (>= 128 mentions + span rows)


def _build_nc(nchunk=NCHUNK_DEF, debug=False):
    import concourse.bass as bass
    import concourse.mybir as mybir
    import concourse.tile as tile
    from concourse import bacc

    f32 = mybir.dt.float32
    bf16 = mybir.dt.bfloat16
    fp8 = mybir.dt.float8e4  # e4m3
    i32 = mybir.dt.int32
    ts, ds = bass.ts, bass.ds

    nc = bacc.Bacc("TRN2", target_bir_lowering=False, debug=debug)

    att_r = nc.dram_tensor("att_r", [L, HPG * L], fp8, kind="ExternalInput")
    seqb = nc.dram_tensor("seqb", [L, HA], bf16, kind="ExternalInput")
    seqf = nc.dram_tensor("seqf", [L, H], f32, kind="ExternalInput")
    idx_att = nc.dram_tensor("idx_att", [128, nchunk], i32, kind="ExternalInput")
    idx_mem = nc.dram_tensor("idx_mem", [128, 1], i32, kind="ExternalInput")
    maskS = nc.dram_tensor("maskS", [128, nchunk * K], bf16, kind="ExternalInput")
    maskCT = nc.dram_tensor("maskCT", [K, L], f32, kind="ExternalInput")
    ident = nc.dram_tensor("ident", [128, 128], bf16, kind="ExternalInput")
    ident8 = nc.dram_tensor("ident8", [128, 128], fp8, kind="ExternalInput")
    out_mnum = nc.dram_tensor("out_mnum", [EM, HA], bf16, kind="ExternalOutput")
    out_v = nc.dram_tensor("out_v", [4 * K, HA], bf16, kind="ExternalOutput")
    out_memb = nc.dram_tensor("out_memb", [EM, H], f32, kind="ExternalOutput")

    with tile.TileContext(nc) as tc:
        with (
            tc.tile_pool(name="const", bufs=1) as constp,
            tc.tile_pool(name="gat", bufs=3) as gatp,
            tc.tile_pool(name="acc", bufs=1) as accp,
            tc.tile_pool(name="stage", bufs=1) as stagep,
            tc.tile_pool(name="pshold", bufs=1, space="PSUM") as pshold,
            tc.tile_pool(name="psrot", bufs=2, space="PSUM") as psrot,
        ):
            # ---- indices first, on the gpsimd queue itself: no cross-engine
            #      semaphore wait before the first gather's descriptor-gen ----
            idxa_s = constp.tile([128, nchunk], i32, tag="idxa", name="idxa")
            idxm_s = constp.tile([128, 1], i32, tag="idxm", name="idxm")
            nc.sync.dma_start(out=idxa_s[:], in_=idx_att[:])
            nc.sync.dma_start(out=idxm_s[:], in_=idx_mem[:])

            g5s = []
            for q in range(nchunk):
                g5 = gatp.tile([128, HPG * L], fp8, tag="g5", name="g5")
                nc.gpsimd.indirect_dma_start(
                    out=g5[:], out_offset=None, in_=att_r[:],
                    in_offset=bass.IndirectOffsetOnAxis(ap=idxa_s[:, q:q + 1], axis=0))
                g5s.append(g5)

            # ---- mention embeddings: pure f32 row-gather of seq ----
            memb_s = stagep.tile([128, H], f32, tag="memb", name="memb")
            nc.gpsimd.indirect_dma_start(
                out=memb_s[:], out_offset=None, in_=seqf[:],
                in_offset=bass.IndirectOffsetOnAxis(ap=idxm_s[:], axis=0))
            nc.sync.dma_start(out=out_memb[:], in_=memb_s[:])

            # ---- remaining consts; seqb rides the gpsimd queue BEHIND the
            #      gathers so its packets don't race the critical gather ----
            maskS_s = constp.tile([128, nchunk, K], bf16, tag="maskS", name="maskS")
            maskCT_s = constp.tile([K, L], f32, tag="maskCT", name="maskCT")
            ident_s = constp.tile([128, 128], bf16, tag="ident", name="ident")
            ident8_s = constp.tile([128, 128], fp8, tag="ident8", name="ident8")
            seq_s = constp.tile([128, CT, HA], bf16, tag="seqs", name="seqs")
            nc.sync.dma_start(out=maskS_s[:], in_=maskS[:].rearrange("p (q k) -> p q k", k=K))
            nc.scalar.dma_start(out=maskCT_s[:], in_=maskCT[:])
            nc.sync.dma_start(out=ident_s[:], in_=ident[:])
            nc.sync.dma_start(out=ident8_s[:], in_=ident8[:])
            nc.scalar.dma_start(out=seq_s[:], in_=seqb[:].rearrange("(c p) f -> p c f", p=128))

            # ---- head-sum on PE (identity-weight accumulation over 6 heads) ----
            gs_s = accp.tile([128, nchunk, L], bf16, tag="gs", name="gs")
            gtm_s = accp.tile([128, CT, 128], bf16, tag="gtm", name="gtm")
            wv_s = accp.tile([128, CT, K], bf16, tag="wv", name="wv")
            put0 = pshold.tile([K, 512], f32, tag="put0", name="put0")
            put1 = pshold.tile([K, 512], f32, tag="put1", name="put1")
            for q in range(nchunk):
                g5 = g5s[q]
                for half in range(2):
                    ph = psrot.tile([128, 512], f32, tag="ph", name="ph")
                    for h in range(HPG):
                        nc.tensor.matmul(ph[:], ident8_s[:],
                                         g5[:, ds(h * L + half * 512, 512)],
                                         start=(h == 0), stop=(h == HPG - 1))
                    nc.vector.tensor_copy(gs_s[:, q, ds(half * 512, 512)], ph[:])
                # span-row reduction (transposed): uT[k,c] += maskS^T @ gs_q
                nc.tensor.matmul(put0[:], maskS_s[:, q, :], gs_s[:, q, 0:512],
                                 start=(q == 0), stop=(q == nchunk - 1))
                nc.tensor.matmul(put1[:], maskS_s[:, q, :], gs_s[:, q, 512:L],
                                 start=(q == 0), stop=(q == nchunk - 1))
                if q == 0:
                    # mention rows of S: PE-transpose of gathered chunk 0
                    for ct in range(CT):
                        pt = psrot.tile([128, 128], bf16, tag="pt", name="pt")
                        nc.tensor.transpose(pt[:], gs_s[:, 0, ts(ct, 128)], ident_s[:])
                        nc.vector.tensor_copy(gtm_s[:, ct, :], pt[:])

            # ---- mnum: mention-context numerators (runs early, off gtm) ----
            pm0 = pshold.tile([EM, 512], f32, tag="pm0", name="pm0")
            pm1 = pshold.tile([EM, HA - 512], f32, tag="pm1", name="pm1")
            for ct in range(CT):
                st, sp = (ct == 0), (ct == CT - 1)
                nc.tensor.matmul(pm0[:], gtm_s[:, ct, :], seq_s[:, ct, 0:512], start=st, stop=sp)
                nc.tensor.matmul(pm1[:], gtm_s[:, ct, :], seq_s[:, ct, 512:HA], start=st, stop=sp)
            mnum_s = stagep.tile([EM, HA], bf16, tag="mnum", name="mnum")
            nc.scalar.copy(out=mnum_s[:, 0:512], in_=pm0[:])
            nc.scalar.copy(out=mnum_s[:, 512:HA], in_=pm1[:])
            nc.sync.dma_start(out=out_mnum[:], in_=mnum_s[:])

            # ---- wvT = uT * column-mask, transpose back, col-tiled v matmuls ----
            wvt_s = accp.tile([K, L], bf16, tag="wvt", name="wvt")
            nc.vector.tensor_mul(wvt_s[:, 0:512], put0[:], maskCT_s[:, 0:512])
            nc.vector.tensor_mul(wvt_s[:, 512:L], put1[:], maskCT_s[:, 512:L])
            for ct in range(CT):
                ptk = psrot.tile([128, 128], bf16, tag="pt", name="ptk")
                nc.tensor.transpose(ptk[:, 0:K], wvt_s[:, ts(ct, 128)], ident_s[0:K, 0:K])
                nc.vector.tensor_copy(wv_s[:, ct, :], ptk[:, 0:K])
            pv0 = psrot.tile([128, 512], f32, tag="ph", name="pv0")
            pv1 = psrot.tile([128, 512], f32, tag="ph", name="pv1")
            v_s = stagep.tile([128, HA], bf16, tag="v", name="v")
            for ct in range(CT):
                grp = ct // 2
                st, sp = (ct % 2 == 0), (ct % 2 == 1)
                nc.tensor.matmul(pv0[ds(32 * grp, K), :], wv_s[:, ct, :],
                                 seq_s[:, ct, 0:512], start=st, stop=sp,
                                 tile_position=(0, 32 * grp))
                nc.tensor.matmul(pv1[ds(32 * grp, K), 0:HA - 512], wv_s[:, ct, :],
                                 seq_s[:, ct, 512:HA], start=st, stop=sp,
                                 tile_position=(0, 32 * grp))
                if sp:
                    nc.scalar.copy(out=v_s[ds(32 * grp, K), 0:512],
                                   in_=pv0[ds(32 * grp, K), :])
                    nc.vector.tensor_copy(v_s[ds(32 * grp, K), 512:HA],
                                          pv1[ds(32 * grp, K), 0:HA - 512])
                    nc.scalar.dma_start(out=out_v[ts(grp, K), :],
                                        in_=v_s[ds(32 * grp, K), :])

    nc.compile()
    return nc


_NC_CACHE = {}


def _get_nc(nchunk=NCHUNK_DEF):
    if nchunk not in _NC_CACHE:
        _NC_CACHE[nchunk] = _build_nc(nchunk)
    return _NC_CACHE[nchunk]


def _per_core_inputs(sequence_output, attention, mention_pos, link_start, link_len):
    """Returns (in_maps for 8 cores, per-doc span lengths, nchunk)."""
    import ml_dtypes
    seq = np.ascontiguousarray(np.asarray(sequence_output, dtype=np.float32))
    att = np.asarray(attention)
    mpos = np.asarray(mention_pos).astype(np.int64)
    lstart = np.asarray(link_start).astype(np.int64)
    llen = np.asarray(link_len).astype(np.int64)

    docs = []
    max_slots = 0
    for b in range(B):
        pos = (mpos[b] + OFFSET).reshape(EM)
        s = lstart[b] + OFFSET
        e = lstart[b] + llen[b] + 1 + OFFSET
        row2slot = {}
        slots = list(pos)
        for j, r in enumerate(pos):
            row2slot.setdefault(int(r), j)
        for si, ei in zip(s, e):
            for r in range(int(si), int(ei)):
                if r not in row2slot:
                    row2slot[r] = len(slots)
                    slots.append(r)
        docs.append((pos, s, e, row2slot, slots))
        max_slots = max(max_slots, len(slots))
    nchunk = max(NCHUNK_DEF, -(-max_slots // 128))

    ident = np.eye(128, dtype=ml_dtypes.bfloat16)
    ident8 = np.eye(128, dtype=ml_dtypes.float8_e4m3fn)
    in_maps = []
    lengths = []
    for b in range(B):
        pos, s, e, row2slot, slots = docs[b]
        n_slots = nchunk * 128
        sl = np.zeros(n_slots, np.int32)
        sl[:len(slots)] = slots
        idx_att = np.ascontiguousarray(sl.reshape(nchunk, 128).T)
        idx_mem = np.ascontiguousarray(pos.astype(np.int32).reshape(128, 1))
        mS = np.zeros((n_slots, K), np.float32)
        mC = np.zeros((L, K), np.float32)
        for k, (si, ei) in enumerate(zip(s, e)):
            mC[int(si):int(ei), k] = 1.0
            for r in range(int(si), int(ei)):
                mS[row2slot[r], k] = 1.0
        maskS = np.ascontiguousarray(
            mS.reshape(nchunk, 128, K).transpose(1, 0, 2).reshape(128, nchunk * K)
        ).astype(ml_dtypes.bfloat16)
        maskCT = np.ascontiguousarray(mC.T)
        seqb = np.concatenate(
            [seq[b], np.ones((L, 1), np.float32), np.zeros((L, HA - H - 1), np.float32)],
            axis=1).astype(ml_dtypes.bfloat16)
        seqf = seq[b]
        lengths.append((e - s).astype(np.float32))
        for g in range(2):
            att_r = np.ascontiguousarray(
                att[b, g * HPG:(g + 1) * HPG].astype(ml_dtypes.float8_e4m3fn)
                .transpose(1, 0, 2).reshape(L, HPG * L))
            in_maps.append({
                "att_r": att_r, "seqb": seqb, "seqf": seqf,
                "idx_att": idx_att, "idx_mem": idx_mem,
                "maskS": maskS, "maskCT": maskCT, "ident": ident, "ident8": ident8,
            })
    return in_maps, lengths, nchunk


def _combine(outs, lengths, type_table):
    ttab = np.asarray(type_table, dtype=np.float32)
    type_ids = np.concatenate(
        [np.zeros(E, np.int64), np.ones(EM, np.int64), np.full(K, 2, np.int64)])
    nodes_type = ttab[type_ids]  # [E+EM+K, TYPE_DIM]

    out = np.zeros((B, E + EM + K + E + EM, H + TYPE_DIM), np.float32)
    for b in range(B):
        o0, o1 = outs[2 * b], outs[2 * b + 1]
        v4 = np.asarray(o0["out_v"], np.float32) + np.asarray(o1["out_v"], np.float32)
        v = v4.reshape(4, K, HA).sum(axis=0)
        mnum = (np.asarray(o0["out_mnum"], np.float32)
                + np.asarray(o1["out_mnum"], np.float32))
        memb = o0["out_memb"]
        length = lengths[b]

        link_rep = v[:, :H] / (NH * length[:, None])
        m_ctx = mnum[:, :H] / (mnum[:, H:H + 1] + NH * 1e-5)
        enum = mnum.reshape(E, MPE, HA).sum(axis=1)
        e_ctx = enum[:, :H] / (enum[:, H:H + 1] + NH * MPE * 1e-5)

        mg = memb.reshape(E, MPE, H)
        mmax = mg.max(axis=1)
        eemb = np.log(np.exp(mg - mmax[:, None, :]).sum(axis=1)) + mmax

        nodes_raw = np.concatenate([eemb, memb, link_rep], axis=0)      # [176,H]
        nodes = np.concatenate([nodes_raw, nodes_type], axis=1)         # [176,H+20]
        ctx = np.concatenate([e_ctx, m_ctx], axis=0)                    # [160,H]
        ctx = np.concatenate([ctx, np.zeros((E + EM, TYPE_DIM), np.float32)], axis=1)
        out[b] = np.concatenate([nodes, ctx], axis=0)
    return out


def kernel(**inputs):
    from concourse.bass_utils import run_bass_kernel_spmd

    in_maps, lengths, nchunk = _per_core_inputs(
        inputs["sequence_output"], inputs["attention"],
        inputs["mention_pos"], inputs["link_start"], inputs["link_len"])
    nc = _get_nc(nchunk)
    res = run_bass_kernel_spmd(nc, in_maps, core_ids=list(range(8)))
    return _combine(res.results, lengths, inputs["type_table"])


# revision 30
# speedup vs baseline: 1.1174x; 1.0849x over previous
"""Trainium2 Bass kernel for nn_DocREModel (doc-level relation extraction graph pooling).

Strategy (8 NeuronCores): each doc b (B=4) is split across 2 cores by attention
heads (6 heads each).  Key observation: the model only ever reads attention rows
at mention positions (<=128 distinct) and inside link spans, i.e. ~35% of the
[1024,1024] matrix.  Each core therefore device-GATHERS just those rows
(SWDGE indirect DMA with a runtime per-partition index tile), head-sums them on
the vector engine, and runs small PE matmuls:

  - att_r HBM layout [row, head*1024+col] (bf16): gathering one row pulls all 6
    heads of that row contiguously (12 KB/descriptor).
  - gather slot j<128 == mention j (EM=128 exactly), so the "onehot" gather
    matrix is the identity: mention rows of S come from a PE transpose of
    gathered chunk 0.  Span rows occupy slots >=128; a host-built slot mask
    [slots,16] reduces them to per-span row-sums uT[16,L] via PE matmul
    (transposed so each PSUM bank holds a single accumulation group).
  - mnum = S_mention^T @ [seq|1]  (context numerators + row-sums)
  - v    = (uT*maskCT)^T-transposed-back @ [seq|1], col-group-tiled 4x on the
    PE (16-row outputs packed at partition offsets 0/32/64/96; host sums).
  - memb = seq rows at mentions via a second (f32) indirect gather - no compute.
The host applies the tiny normalizations (head-count / span-length / row-sum
divides, entity pooling, 4-way logsumexp) while unsharding.
"""

import os
import sys

for _p in ("/opt/trn_rl_repo", "/root/.axon_site/_ro/trn_rl_repo"):
    if os.path.isdir(_p) and _p not in sys.path:
        sys.path.insert(0, _p)

import numpy as np

B, L, H, NH = 4, 1024, 768, 12
E, MPE, K = 32, 4, 16
EM = E * MPE              # 128 mentions per doc == gather chunk 0
TYPE_DIM = 20
OFFSET = 1
HPG = NH // 2             # heads per core (2 cores per doc)
CT = L // 128             # 8 column chunks
HA = H + 4                # seq | ones | 3 zero-pad -> 772 (row-sum in col 768)
NCHUNK_DEF = 3            # gather slots = 128*NCHUNK (>= 128 mentions + span rows)


def _build_nc(nchunk=NCHUNK_DEF, debug=False):
    import concourse.bass as bass
    import concourse.mybir as mybir
    import concourse.tile as tile
    from concourse import bacc

    f32 = mybir.dt.float32
    bf16 = mybir.dt.bfloat16
    fp8 = mybir.dt.float8e4  # e4m3
    i32 = mybir.dt.int32
    ts, ds = bass.ts, bass.ds

    nc = bacc.Bacc("TRN2", target_bir_lowering=False, debug=debug)

    att_r = nc.dram_tensor("att_r", [L, HPG * L], fp8, kind="ExternalInput")
    seqb = nc.dram_tensor("seqb", [L, HA], bf16, kind="ExternalInput")
    seqf = nc.dram_tensor("seqf", [L, H], f32, kind="ExternalInput")
    idx_att = nc.dram_tensor("idx_att", [128, nchunk], i32, kind="ExternalInput")
    idx_mem = nc.dram_tensor("idx_mem", [128, 1], i32, kind="ExternalInput")
    maskS = nc.dram_tensor("maskS", [128, nchunk * K], bf16, kind="ExternalInput")
    maskCT = nc.dram_tensor("maskCT", [K, L], f32, kind="ExternalInput")
    ident = nc.dram_tensor("ident", [128, 128], bf16, kind="ExternalInput")
    ident8 = nc.dram_tensor("ident8", [128, 128], fp8, kind="ExternalInput")
    out_mnum = nc.dram_tensor("out_mnum", [EM, HA], bf16, kind="ExternalOutput")
    out_v = nc.dram_tensor("out_v", [4 * K, HA], bf16, kind="ExternalOutput")
    out_memb = nc.dram_tensor("out_memb", [EM, H], f32, kind="ExternalOutput")

    with tile.TileContext(nc) as tc:
        with (
            tc.tile_pool(name="const", bufs=1) as constp,
            tc.tile_pool(name="gat", bufs=3) as gatp,
            tc.tile_pool(name="acc", bufs=1) as accp,
            tc.tile_pool(name="stage", bufs=1) as stagep,
            tc.tile_pool(name="pshold", bufs=1, space="PSUM") as pshold,
            tc.tile_pool(name="psrot", bufs=2, space="PSUM") as psrot,
        ):
            # ---- indices first, on the gpsimd queue itself: no cross-engine
            #      semaphore wait before the first gather's descriptor-gen ----
            idxa_s = constp.tile([128, nchunk], i32, tag="idxa", name="idxa")
            idxm_s = constp.tile([128, 1], i32, tag="idxm", name="idxm")
            nc.sync.dma_start(out=idxa_s[:], in_=idx_att[:])
            nc.sync.dma_start(out=idxm_s[:], in_=idx_mem[:])

            g5s = []
            for q in range(nchunk):
                g5 = gatp.tile([128, HPG * L], fp8, tag="g5", name="g5")
                nc.gpsimd.indirect_dma_start(
                    out=g5[:], out_offset=None, in_=att_r[:],
                    in_offset=bass.IndirectOffsetOnAxis(ap=idxa_s[:, q:q + 1], axis=0))
                g5s.append(g5)

            # ---- mention embeddings: pure f32 row-gather of seq ----
            memb_s = stagep.tile([128, H], f32, tag="memb", name="memb")
            nc.gpsimd.indirect_dma_start(
                out=memb_s[:], out_offset=None, in_=seqf[:],
                in_offset=bass.IndirectOffsetOnAxis(ap=idxm_s[:], axis=0))
            nc.sync.dma_start(out=out_memb[:], in_=memb_s[:])

            # ---- remaining consts; seqb rides the gpsimd queue BEHIND the
            #      gathers so its packets don't race the critical gather ----
            maskS_s = constp.tile([128, nchunk, K], bf16, tag="maskS", name="maskS")
            maskCT_s = constp.tile([K, L], f32, tag="maskCT", name="maskCT")
            ident_s = constp.tile([128, 128], bf16, tag="ident", name="ident")
            ident8_s = constp.tile([128, 128], fp8, tag="ident8", name="ident8")
            seq_s = constp.tile([128, CT, HA], bf16, tag="seqs", name="seqs")
            nc.sync.dma_start(out=maskS_s[:], in_=maskS[:].rearrange("p (q k) -> p q k", k=K))
            nc.scalar.dma_start(out=maskCT_s[:], in_=maskCT[:])
            nc.sync.dma_start(out=ident_s[:], in_=ident[:])
            nc.sync.dma_start(out=ident8_s[:], in_=ident8[:])
            nc.scalar.dma_start(out=seq_s[:], in_=seqb[:].rearrange("(c p) f -> p c f", p=128))

            # ---- head-sum on PE (identity-weight accumulation over 6 heads) ----
            gs_s = accp.tile([128, nchunk, L], bf16, tag="gs", name="gs")
            gtm_s = accp.tile([128, CT, 128], bf16, tag="gtm", name="gtm")
            wv_s = accp.tile([128, CT, K], bf16, tag="wv", name="wv")
            put0 = pshold.tile([K, 512], f32, tag="put0", name="put0")
            put1 = pshold.tile([K, 512], f32, tag="put1", name="put1")
            for q in range(nchunk):
                g5 = g5s[q]
                for half in range(2):
                    ph = psrot.tile([128, 512], f32, tag="ph", name="ph")
                    for h in range(HPG):
                        nc.tensor.matmul(ph[:], ident8_s[:],
                                         g5[:, ds(h * L + half * 512, 512)],
                                         start=(h == 0), stop=(h == HPG - 1))
                    nc.scalar.copy(out=gs_s[:, q, ds(half * 512, 512)], in_=ph[:])
                # span-row reduction (transposed): uT[k,c] += maskS^T @ gs_q
                nc.tensor.matmul(put0[:], maskS_s[:, q, :], gs_s[:, q, 0:512],
                                 start=(q == 0), stop=(q == nchunk - 1))
                nc.tensor.matmul(put1[:], maskS_s[:, q, :], gs_s[:, q, 512:L],
                                 start=(q == 0), stop=(q == nchunk - 1))
                if q == 0:
                    # mention rows of S: PE-transpose of gathered chunk 0
                    for ct in range(CT):
                        pt = psrot.tile([128, 128], bf16, tag="pt", name="pt")
                        nc.tensor.transpose(pt[:], gs_s[:, 0, ts(ct, 128)], ident_s[:])
                        nc.vector.tensor_copy(gtm_s[:, ct, :], pt[:])

            # ---- mnum: mention-context numerators (runs early, off gtm) ----
            pm0 = pshold.tile([EM, 512], f32, tag="pm0", name="pm0")
            pm1 = pshold.tile([EM, HA - 512], f32, tag="pm1", name="pm1")
            for ct in range(CT):
                st, sp = (ct == 0), (ct == CT - 1)
                nc.tensor.matmul(pm0[:], gtm_s[:, ct, :], seq_s[:, ct, 0:512], start=st, stop=sp)
                nc.tensor.matmul(pm1[:], gtm_s[:, ct, :], seq_s[:, ct, 512:HA], start=st, stop=sp)
            mnum_s = stagep.tile([EM, HA], bf16, tag="mnum", name="mnum")
            nc.scalar.copy(out=mnum_s[:, 0:512], in_=pm0[:])
            nc.scalar.copy(out=mnum_s[:, 512:HA], in_=pm1[:])
            nc.sync.dma_start(out=out_mnum[:], in_=mnum_s[:])

            # ---- wvT = uT * column-mask, transpose back, col-tiled v matmuls ----
            wvt_s = accp.tile([K, L], bf16, tag="wvt", name="wvt")
            nc.vector.tensor_mul(wvt_s[:, 0:512], put0[:], maskCT_s[:, 0:512])
            nc.vector.tensor_mul(wvt_s[:, 512:L], put1[:], maskCT_s[:, 512:L])
            for ct in range(CT):
                ptk = psrot.tile([128, 128], bf16, tag="pt", name="ptk")
                nc.tensor.transpose(ptk[:, 0:K], wvt_s[:, ts(ct, 128)], ident_s[0:K, 0:K])
                nc.vector.tensor_copy(wv_s[:, ct, :], ptk[:, 0:K])
            pv0 = psrot.tile([128, 512], f32, tag="ph", name="pv0")
            pv1 = psrot.tile([128, 512], f32, tag="ph", name="pv1")
            v_s = stagep.tile([128, HA], bf16, tag="v", name="v")
            for ct in range(CT):
                grp = ct // 2
                st, sp = (ct % 2 == 0), (ct % 2 == 1)
                nc.tensor.matmul(pv0[ds(32 * grp, K), :], wv_s[:, ct, :],
                                 seq_s[:, ct, 0:512], start=st, stop=sp,
                                 tile_position=(0, 32 * grp))
                nc.tensor.matmul(pv1[ds(32 * grp, K), 0:HA - 512], wv_s[:, ct, :],
                                 seq_s[:, ct, 512:HA], start=st, stop=sp,
                                 tile_position=(0, 32 * grp))
            for grp in range(4):
                nc.scalar.copy(out=v_s[ds(32 * grp, K), 0:512],
                               in_=pv0[ds(32 * grp, K), :])
                nc.vector.tensor_copy(v_s[ds(32 * grp, K), 512:HA],
                                      pv1[ds(32 * grp, K), 0:HA - 512])
            for grp in range(4):
                nc.scalar.dma_start(out=out_v[ts(grp, K), :], in_=v_s[ds(32 * grp, K), :])

    nc.compile()
    return nc


_NC_CACHE = {}


def _get_nc(nchunk=NCHUNK_DEF):
    if nchunk not in _NC_CACHE:
        _NC_CACHE[nchunk] = _build_nc(nchunk)
    return _NC_CACHE[nchunk]


def _per_core_inputs(sequence_output, attention, mention_pos, link_start, link_len):
    """Returns (in_maps for 8 cores, per-doc span lengths, nchunk)."""
    import ml_dtypes
    seq = np.ascontiguousarray(np.asarray(sequence_output, dtype=np.float32))
    att = np.asarray(attention)
    mpos = np.asarray(mention_pos).astype(np.int64)
    lstart = np.asarray(link_start).astype(np.int64)
    llen = np.asarray(link_len).astype(np.int64)

    docs = []
    max_slots = 0
    for b in range(B):
        pos = (mpos[b] + OFFSET).reshape(EM)
        s = lstart[b] + OFFSET
        e = lstart[b] + llen[b] + 1 + OFFSET
        row2slot = {}
        slots = list(pos)
        for j, r in enumerate(pos):
            row2slot.setdefault(int(r), j)
        for si, ei in zip(s, e):
            for r in range(int(si), int(ei)):
                if r not in row2slot:
                    row2slot[r] = len(slots)
                    slots.append(r)
        docs.append((pos, s, e, row2slot, slots))
        max_slots = max(max_slots, len(slots))
    nchunk = max(NCHUNK_DEF, -(-max_slots // 128))

    ident = np.eye(128, dtype=ml_dtypes.bfloat16)
    ident8 = np.eye(128, dtype=ml_dtypes.float8_e4m3fn)
    in_maps = []
    lengths = []
    for b in range(B):
        pos, s, e, row2slot, slots = docs[b]
        n_slots = nchunk * 128
        sl = np.zeros(n_slots, np.int32)
        sl[:len(slots)] = slots
        idx_att = np.ascontiguousarray(sl.reshape(nchunk, 128).T)
        idx_mem = np.ascontiguousarray(pos.astype(np.int32).reshape(128, 1))
        mS = np.zeros((n_slots, K), np.float32)
        mC = np.zeros((L, K), np.float32)
        for k, (si, ei) in enumerate(zip(s, e)):
            mC[int(si):int(ei), k] = 1.0
            for r in range(int(si), int(ei)):
                mS[row2slot[r], k] = 1.0
        maskS = np.ascontiguousarray(
            mS.reshape(nchunk, 128, K).transpose(1, 0, 2).reshape(128, nchunk * K)
        ).astype(ml_dtypes.bfloat16)
        maskCT = np.ascontiguousarray(mC.T)
        seqb = np.concatenate(
            [seq[b], np.ones((L, 1), np.float32), np.zeros((L, HA - H - 1), np.float32)],
            axis=1).astype(ml_dtypes.bfloat16)
        seqf = seq[b]
        lengths.append((e - s).astype(np.float32))
        for g in range(2):
            att_r = np.ascontiguousarray(
                att[b, g * HPG:(g + 1) * HPG].astype(ml_dtypes.float8_e4m3fn)
                .transpose(1, 0, 2).reshape(L, HPG * L))
            in_maps.append({
                "att_r": att_r, "seqb": seqb, "seqf": seqf,
                "idx_att": idx_att, "idx_mem": idx_mem,
                "maskS": maskS, "maskCT": maskCT, "ident": ident, "ident8": ident8,
            })
    return in_maps, lengths, nchunk


def _combine(outs, lengths, type_table):
    ttab = np.asarray(type_table, dtype=np.float32)
    type_ids = np.concatenate(
        [np.zeros(E, np.int64), np.ones(EM, np.int64), np.full(K, 2, np.int64)])
    nodes_type = ttab[type_ids]  # [E+EM+K, TYPE_DIM]

    out = np.zeros((B, E + EM + K + E + EM, H + TYPE_DIM), np.float32)
    for b in range(B):
        o0, o1 = outs[2 * b], outs[2 * b + 1]
        v4 = np.asarray(o0["out_v"], np.float32) + np.asarray(o1["out_v"], np.float32)
        v = v4.reshape(4, K, HA).sum(axis=0)
        mnum = (np.asarray(o0["out_mnum"], np.float32)
                + np.asarray(o1["out_mnum"], np.float32))
        memb = o0["out_memb"]
        length = lengths[b]

        link_rep = v[:, :H] / (NH * length[:, None])
        m_ctx = mnum[:, :H] / (mnum[:, H:H + 1] + NH * 1e-5)
        enum = mnum.reshape(E, MPE, HA).sum(axis=1)
        e_ctx = enum[:, :H] / (enum[:, H:H + 1] + NH * MPE * 1e-5)

        mg = memb.reshape(E, MPE, H)
        mmax = mg.max(axis=1)
        eemb = np.log(np.exp(mg - mmax[:, None, :]).sum(axis=1)) + mmax

        nodes_raw = np.concatenate([eemb, memb, link_rep], axis=0)      # [176,H]
        nodes = np.concatenate([nodes_raw, nodes_type], axis=1)         # [176,H+20]
        ctx = np.concatenate([e_ctx, m_ctx], axis=0)                    # [160,H]
        ctx = np.concatenate([ctx, np.zeros((E + EM, TYPE_DIM), np.float32)], axis=1)
        out[b] = np.concatenate([nodes, ctx], axis=0)
    return out


def kernel(**inputs):
    from concourse.bass_utils import run_bass_kernel_spmd

    in_maps, lengths, nchunk = _per_core_inputs(
        inputs["sequence_output"], inputs["attention"],
        inputs["mention_pos"], inputs["link_start"], inputs["link_len"])
    nc = _get_nc(nchunk)
    res = run_bass_kernel_spmd(nc, in_maps, core_ids=list(range(8)))
    return _combine(res.results, lengths, inputs["type_table"])


# revision 36
# speedup vs baseline: 1.1847x; 1.0602x over previous
"""Trainium2 Bass kernel for nn_DocREModel (doc-level relation extraction graph pooling).

Strategy (8 NeuronCores): each doc b (B=4) is split across 2 cores by attention
heads (6 heads each).  Key observation: the model only ever reads attention rows
at mention positions (<=128 distinct) and inside link spans, i.e. ~35% of the
[1024,1024] matrix.  Each core therefore device-GATHERS just those rows
(SWDGE indirect DMA with a runtime per-partition index tile), head-sums them on
the vector engine, and runs small PE matmuls:

  - att_r HBM layout [row, head*1024+col] (bf16): gathering one row pulls all 6
    heads of that row contiguously (12 KB/descriptor).
  - gather slot j<128 == mention j (EM=128 exactly), so the "onehot" gather
    matrix is the identity: mention rows of S come from a PE transpose of
    gathered chunk 0.  Span rows occupy slots >=128; a host-built slot mask
    [slots,16] reduces them to per-span row-sums uT[16,L] via PE matmul
    (transposed so each PSUM bank holds a single accumulation group).
  - mnum = S_mention^T @ [seq|1]  (context numerators + row-sums)
  - v    = (uT*maskCT)^T-transposed-back @ [seq|1], col-group-tiled 4x on the
    PE (16-row outputs packed at partition offsets 0/32/64/96; host sums).
  - memb = seq rows at mentions via a second (f32) indirect gather - no compute.
The host applies the tiny normalizations (head-count / span-length / row-sum
divides, entity pooling, 4-way logsumexp) while unsharding.
"""

import os
import sys

for _p in ("/opt/trn_rl_repo", "/root/.axon_site/_ro/trn_rl_repo"):
    if os.path.isdir(_p) and _p not in sys.path:
        sys.path.insert(0, _p)

import numpy as np

B, L, H, NH = 4, 1024, 768, 12
E, MPE, K = 32, 4, 16
EM = E * MPE              # 128 mentions per doc == gather chunk 0
TYPE_DIM = 20
OFFSET = 1
HPG = NH // 2             # heads per core (2 cores per doc)
CT = L // 128             # 8 column chunks
HA = H + 4                # seq | ones | 3 zero-pad -> 772 (row-sum in col 768)
NCHUNK_DEF = 3            # gather slots = 128*NCHUNK (>= 128 mentions + span rows)


def _build_nc(nchunk=NCHUNK_DEF, debug=False):
    import concourse.bass as bass
    import concourse.mybir as mybir
    import concourse.tile as tile
    from concourse import bacc

    f32 = mybir.dt.float32
    bf16 = mybir.dt.bfloat16
    fp8 = mybir.dt.float8e4  # e4m3
    i32 = mybir.dt.int32
    ts, ds = bass.ts, bass.ds

    nc = bacc.Bacc("TRN2", target_bir_lowering=False, debug=debug)

    att_r = nc.dram_tensor("att_r", [L, HPG * L], fp8, kind="ExternalInput")
    seqb = nc.dram_tensor("seqb", [L, HA], bf16, kind="ExternalInput")
    seqf = nc.dram_tensor("seqf", [L, H], f32, kind="ExternalInput")
    idx_att = nc.dram_tensor("idx_att", [128, nchunk], i32, kind="ExternalInput")
    idx_mem = nc.dram_tensor("idx_mem", [128, 1], i32, kind="ExternalInput")
    maskS = nc.dram_tensor("maskS", [128, nchunk * K], bf16, kind="ExternalInput")
    maskCT = nc.dram_tensor("maskCT", [K, L], f32, kind="ExternalInput")
    ident = nc.dram_tensor("ident", [128, 128], bf16, kind="ExternalInput")
    ident82 = nc.dram_tensor("ident82", [128, 256], fp8, kind="ExternalInput")
    out_mnum = nc.dram_tensor("out_mnum", [EM, HA], bf16, kind="ExternalOutput")
    out_v = nc.dram_tensor("out_v", [4 * K, HA], bf16, kind="ExternalOutput")
    out_memb = nc.dram_tensor("out_memb", [EM, H], f32, kind="ExternalOutput")

    with tile.TileContext(nc) as tc:
        with (
            tc.tile_pool(name="const", bufs=1) as constp,
            tc.tile_pool(name="gat", bufs=3) as gatp,
            tc.tile_pool(name="acc", bufs=1) as accp,
            tc.tile_pool(name="stage", bufs=1) as stagep,
            tc.tile_pool(name="pshold", bufs=1, space="PSUM") as pshold,
            tc.tile_pool(name="psrot", bufs=2, space="PSUM") as psrot,
        ):
            # ---- indices first, on the gpsimd queue itself: no cross-engine
            #      semaphore wait before the first gather's descriptor-gen ----
            idxa_s = constp.tile([128, nchunk], i32, tag="idxa", name="idxa")
            idxm_s = constp.tile([128, 1], i32, tag="idxm", name="idxm")
            nc.sync.dma_start(out=idxa_s[:], in_=idx_att[:])
            nc.sync.dma_start(out=idxm_s[:], in_=idx_mem[:])

            g5s = []
            for q in range(nchunk):
                g5 = gatp.tile([128, HPG * L], fp8, tag="g5", name="g5")
                nc.gpsimd.indirect_dma_start(
                    out=g5[:], out_offset=None, in_=att_r[:],
                    in_offset=bass.IndirectOffsetOnAxis(ap=idxa_s[:, q:q + 1], axis=0))
                g5s.append(g5)

            # ---- mention embeddings: pure f32 row-gather of seq ----
            memb_s = stagep.tile([128, H], f32, tag="memb", name="memb")
            nc.gpsimd.indirect_dma_start(
                out=memb_s[:], out_offset=None, in_=seqf[:],
                in_offset=bass.IndirectOffsetOnAxis(ap=idxm_s[:], axis=0))
            nc.sync.dma_start(out=out_memb[:], in_=memb_s[:])

            # ---- remaining consts; seqb rides the gpsimd queue BEHIND the
            #      gathers so its packets don't race the critical gather ----
            maskS_s = constp.tile([128, nchunk, K], bf16, tag="maskS", name="maskS")
            maskCT_s = constp.tile([K, L], f32, tag="maskCT", name="maskCT")
            ident_s = constp.tile([128, 128], bf16, tag="ident", name="ident")
            ident82_s = constp.tile([128, 256], fp8, tag="ident82", name="ident82")
            seq_s = constp.tile([128, CT, HA], bf16, tag="seqs", name="seqs")
            nc.sync.dma_start(out=maskS_s[:], in_=maskS[:].rearrange("p (q k) -> p q k", k=K))
            nc.scalar.dma_start(out=maskCT_s[:], in_=maskCT[:])
            nc.sync.dma_start(out=ident_s[:], in_=ident[:])
            nc.sync.dma_start(out=ident82_s[:], in_=ident82[:])
            nc.scalar.dma_start(out=seq_s[:], in_=seqb[:].rearrange("(c p) f -> p c f", p=128))

            # ---- head-sum on PE (identity-weight accumulation over 6 heads) ----
            gs_s = accp.tile([128, nchunk, L], bf16, tag="gs", name="gs")
            gtm_s = accp.tile([128, CT, 128], bf16, tag="gtm", name="gtm")
            wv_s = accp.tile([128, CT, K], bf16, tag="wv", name="wv")
            put0 = pshold.tile([K, 512], f32, tag="put0", name="put0")
            put1 = pshold.tile([K, 512], f32, tag="put1", name="put1")
            for q in range(nchunk):
                g5r = g5s[q][:].rearrange("p (h c) -> p h c", h=HPG)
                for half in range(2):
                    ph = psrot.tile([128, 512], f32, tag="ph", name="ph")
                    for hp in range(HPG // 2):
                        nc.tensor.matmul(
                            ph[:], ident82_s[:].rearrange("p (two m) -> p two m", two=2),
                            g5r[:, ds(2 * hp, 2), ds(half * 512, 512)],
                            start=(hp == 0), stop=(hp == HPG // 2 - 1),
                            perf_mode=mybir.MatmulPerfMode.DoubleRow)
                    if half == 0:
                        nc.scalar.copy(out=gs_s[:, q, ds(half * 512, 512)], in_=ph[:])
                    else:
                        nc.vector.tensor_copy(gs_s[:, q, ds(half * 512, 512)], ph[:])
                # span-row reduction (transposed): uT[k,c] += maskS^T @ gs_q
                nc.tensor.matmul(put0[:], maskS_s[:, q, :], gs_s[:, q, 0:512],
                                 start=(q == 0), stop=(q == nchunk - 1))
                nc.tensor.matmul(put1[:], maskS_s[:, q, :], gs_s[:, q, 512:L],
                                 start=(q == 0), stop=(q == nchunk - 1))
                if q == 0:
                    # mention rows of S: PE-transpose of gathered chunk 0
                    for ct in range(CT):
                        pt = psrot.tile([128, 128], bf16, tag="pt", name="pt")
                        nc.tensor.transpose(pt[:], gs_s[:, 0, ts(ct, 128)], ident_s[:])
                        nc.vector.tensor_copy(gtm_s[:, ct, :], pt[:])

            # ---- mnum: mention-context numerators (runs early, off gtm) ----
            pm0 = pshold.tile([EM, 512], f32, tag="pm0", name="pm0")
            pm1 = pshold.tile([EM, HA - 512], f32, tag="pm1", name="pm1")
            for ct in range(CT):
                st, sp = (ct == 0), (ct == CT - 1)
                nc.tensor.matmul(pm0[:], gtm_s[:, ct, :], seq_s[:, ct, 0:512], start=st, stop=sp)
                nc.tensor.matmul(pm1[:], gtm_s[:, ct, :], seq_s[:, ct, 512:HA], start=st, stop=sp)
            mnum_s = stagep.tile([EM, HA], bf16, tag="mnum", name="mnum")
            nc.scalar.copy(out=mnum_s[:, 0:512], in_=pm0[:])
            nc.scalar.copy(out=mnum_s[:, 512:HA], in_=pm1[:])
            nc.sync.dma_start(out=out_mnum[:], in_=mnum_s[:])

            # ---- wvT = uT * column-mask, transpose back, col-tiled v matmuls ----
            wvt_s = accp.tile([K, L], bf16, tag="wvt", name="wvt")
            nc.vector.tensor_mul(wvt_s[:, 0:512], put0[:], maskCT_s[:, 0:512])
            nc.vector.tensor_mul(wvt_s[:, 512:L], put1[:], maskCT_s[:, 512:L])
            for ct in range(CT):
                ptk = psrot.tile([128, 128], bf16, tag="pt", name="ptk")
                nc.tensor.transpose(ptk[:, 0:K], wvt_s[:, ts(ct, 128)], ident_s[0:K, 0:K])
                nc.vector.tensor_copy(wv_s[:, ct, :], ptk[:, 0:K])
            pv0 = psrot.tile([128, 512], f32, tag="ph", name="pv0")
            pv1 = psrot.tile([128, 512], f32, tag="ph", name="pv1")
            v_s = stagep.tile([128, HA], bf16, tag="v", name="v")
            for ct in range(CT):
                grp = ct // 2
                st, sp = (ct % 2 == 0), (ct % 2 == 1)
                nc.tensor.matmul(pv0[ds(32 * grp, K), :], wv_s[:, ct, :],
                                 seq_s[:, ct, 0:512], start=st, stop=sp,
                                 tile_position=(0, 32 * grp))
                nc.tensor.matmul(pv1[ds(32 * grp, K), 0:HA - 512], wv_s[:, ct, :],
                                 seq_s[:, ct, 512:HA], start=st, stop=sp,
                                 tile_position=(0, 32 * grp))
            for grp in range(4):
                nc.scalar.copy(out=v_s[ds(32 * grp, K), 0:512],
                               in_=pv0[ds(32 * grp, K), :])
                nc.vector.tensor_copy(v_s[ds(32 * grp, K), 512:HA],
                                      pv1[ds(32 * grp, K), 0:HA - 512])
            for grp in range(4):
                nc.sync.dma_start(out=out_v[ts(grp, K), :], in_=v_s[ds(32 * grp, K), :])

    nc.compile()
    return nc


_NC_CACHE = {}


def _get_nc(nchunk=NCHUNK_DEF):
    if nchunk not in _NC_CACHE:
        _NC_CACHE[nchunk] = _build_nc(nchunk)
    return _NC_CACHE[nchunk]


def _per_core_inputs(sequence_output, attention, mention_pos, link_start, link_len):
    """Returns (in_maps for 8 cores, per-doc span lengths, nchunk)."""
    import ml_dtypes
    seq = np.ascontiguousarray(np.asarray(sequence_output, dtype=np.float32))
    att = np.asarray(attention)
    mpos = np.asarray(mention_pos).astype(np.int64)
    lstart = np.asarray(link_start).astype(np.int64)
    llen = np.asarray(link_len).astype(np.int64)

    docs = []
    max_slots = 0
    for b in range(B):
        pos = (mpos[b] + OFFSET).reshape(EM)
        s = lstart[b] + OFFSET
        e = lstart[b] + llen[b] + 1 + OFFSET
        row2slot = {}
        slots = list(pos)
        for j, r in enumerate(pos):
            row2slot.setdefault(int(r), j)
        for si, ei in zip(s, e):
            for r in range(int(si), int(ei)):
                if r not in row2slot:
                    row2slot[r] = len(slots)
                    slots.append(r)
        docs.append((pos, s, e, row2slot, slots))
        max_slots = max(max_slots, len(slots))
    nchunk = max(NCHUNK_DEF, -(-max_slots // 128))

    ident = np.eye(128, dtype=ml_dtypes.bfloat16)
    i8 = np.eye(128, dtype=ml_dtypes.float8_e4m3fn)
    ident82 = np.ascontiguousarray(np.concatenate([i8, i8], axis=1))
    in_maps = []
    lengths = []
    for b in range(B):
        pos, s, e, row2slot, slots = docs[b]
        n_slots = nchunk * 128
        sl = np.zeros(n_slots, np.int32)
        sl[:len(slots)] = slots
        idx_att = np.ascontiguousarray(sl.reshape(nchunk, 128).T)
        idx_mem = np.ascontiguousarray(pos.astype(np.int32).reshape(128, 1))
        mS = np.zeros((n_slots, K), np.float32)
        mC = np.zeros((L, K), np.float32)
        for k, (si, ei) in enumerate(zip(s, e)):
            mC[int(si):int(ei), k] = 1.0
            for r in range(int(si), int(ei)):
                mS[row2slot[r], k] = 1.0
        maskS = np.ascontiguousarray(
            mS.reshape(nchunk, 128, K).transpose(1, 0, 2).reshape(128, nchunk * K)
        ).astype(ml_dtypes.bfloat16)
        maskCT = np.ascontiguousarray(mC.T)
        seqb = np.concatenate(
            [seq[b], np.ones((L, 1), np.float32), np.zeros((L, HA - H - 1), np.float32)],
            axis=1).astype(ml_dtypes.bfloat16)
        seqf = seq[b]
        lengths.append((e - s).astype(np.float32))
        for g in range(2):
            att_r = np.ascontiguousarray(
                att[b, g * HPG:(g + 1) * HPG].astype(ml_dtypes.float8_e4m3fn)
                .transpose(1, 0, 2).reshape(L, HPG * L))
            in_maps.append({
                "att_r": att_r, "seqb": seqb, "seqf": seqf,
                "idx_att": idx_att, "idx_mem": idx_mem,
                "maskS": maskS, "maskCT": maskCT, "ident": ident, "ident82": ident82,
            })
    return in_maps, lengths, nchunk


def _combine(outs, lengths, type_table):
    ttab = np.asarray(type_table, dtype=np.float32)
    type_ids = np.concatenate(
        [np.zeros(E, np.int64), np.ones(EM, np.int64), np.full(K, 2, np.int64)])
    nodes_type = ttab[type_ids]  # [E+EM+K, TYPE_DIM]

    out = np.zeros((B, E + EM + K + E + EM, H + TYPE_DIM), np.float32)
    for b in range(B):
        o0, o1 = outs[2 * b], outs[2 * b + 1]
        v4 = np.asarray(o0["out_v"], np.float32) + np.asarray(o1["out_v"], np.float32)
        v = v4.reshape(4, K, HA).sum(axis=0)
        mnum = (np.asarray(o0["out_mnum"], np.float32)
                + np.asarray(o1["out_mnum"], np.float32))
        memb = o0["out_memb"]
        length = lengths[b]

        link_rep = v[:, :H] / (NH * length[:, None])
        m_ctx = mnum[:, :H] / (mnum[:, H:H + 1] + NH * 1e-5)
        enum = mnum.reshape(E, MPE, HA).sum(axis=1)
        e_ctx = enum[:, :H] / (enum[:, H:H + 1] + NH * MPE * 1e-5)

        mg = memb.reshape(E, MPE, H)
        mmax = mg.max(axis=1)
        eemb = np.log(np.exp(mg - mmax[:, None, :]).sum(axis=1)) + mmax

        nodes_raw = np.concatenate([eemb, memb, link_rep], axis=0)      # [176,H]
        nodes = np.concatenate([nodes_raw, nodes_type], axis=1)         # [176,H+20]
        ctx = np.concatenate([e_ctx, m_ctx], axis=0)                    # [160,H]
        ctx = np.concatenate([ctx, np.zeros((E + EM, TYPE_DIM), np.float32)], axis=1)
        out[b] = np.concatenate([nodes, ctx], axis=0)
    return out


def kernel(**inputs):
    from concourse.bass_utils import run_bass_kernel_spmd

    in_maps, lengths, nchunk = _per_core_inputs(
        inputs["sequence_output"], inputs["attention"],
        inputs["mention_pos"], inputs["link_start"], inputs["link_len"])
    nc = _get_nc(nchunk)
    res = run_bass_kernel_spmd(nc, in_maps, core_ids=list(range(8)))
    return _combine(res.results, lengths, inputs["type_table"])


# revision 45
# speedup vs baseline: 1.2149x; 1.0255x over previous
"""Trainium2 Bass kernel for nn_DocREModel (doc-level relation extraction graph pooling).

Strategy (8 NeuronCores): each doc b (B=4) is split across 2 cores by attention
heads (6 heads each).  Key observation: the model only ever reads attention rows
at mention positions (<=128 distinct) and inside link spans, i.e. ~35% of the
[1024,1024] matrix.  Each core therefore device-GATHERS just those rows
(SWDGE indirect DMA with a runtime per-partition index tile), head-sums them on
the vector engine, and runs small PE matmuls:

  - att_r HBM layout [row, head*1024+col] (bf16): gathering one row pulls all 6
    heads of that row contiguously (12 KB/descriptor).
  - gather slot j<128 == mention j (EM=128 exactly), so the "onehot" gather
    matrix is the identity: mention rows of S come from a PE transpose of
    gathered chunk 0.  Span rows occupy slots >=128; a host-built slot mask
    [slots,16] reduces them to per-span row-sums uT[16,L] via PE matmul
    (transposed so each PSUM bank holds a single accumulation group).
  - mnum = S_mention^T @ [seq|1]  (context numerators + row-sums)
  - v    = (uT*maskCT)^T-transposed-back @ [seq|1], col-group-tiled 4x on the
    PE (16-row outputs packed at partition offsets 0/32/64/96; host sums).
  - memb = seq rows at mentions via a second (f32) indirect gather - no compute.
The host applies the tiny normalizations (head-count / span-length / row-sum
divides, entity pooling, 4-way logsumexp) while unsharding.
"""

import os
import sys

for _p in ("/opt/trn_rl_repo", "/root/.axon_site/_ro/trn_rl_repo"):
    if os.path.isdir(_p) and _p not in sys.path:
        sys.path.insert(0, _p)

import numpy as np

B, L, H, NH = 4, 1024, 768, 12
E, MPE, K = 32, 4, 16
EM = E * MPE              # 128 mentions per doc == gather chunk 0
TYPE_DIM = 20
OFFSET = 1
HPG = NH // 2             # heads per core (2 cores per doc)
CT = L // 128             # 8 column chunks
HA = H + 4                # seq | ones | 3 zero-pad -> 772 (row-sum in col 768)
NCHUNK_DEF = 3            # gather slots = 128*NCHUNK (>= 128 mentions + span rows)


def _build_nc(nchunk=NCHUNK_DEF, debug=False):
    import concourse.bass as bass
    import concourse.mybir as mybir
    import concourse.tile as tile
    from concourse import bacc

    f32 = mybir.dt.float32
    bf16 = mybir.dt.bfloat16
    fp8 = mybir.dt.float8e4  # e4m3
    i32 = mybir.dt.int32
    ts, ds = bass.ts, bass.ds

    nc = bacc.Bacc("TRN2", target_bir_lowering=False, debug=debug)

    att_r = nc.dram_tensor("att_r", [L, HPG * L], fp8, kind="ExternalInput")
    seqb = nc.dram_tensor("seqb", [L, HA], bf16, kind="ExternalInput")
    idx_att = nc.dram_tensor("idx_att", [128, nchunk], i32, kind="ExternalInput")
    maskS = nc.dram_tensor("maskS", [128, nchunk * K], bf16, kind="ExternalInput")
    maskCT = nc.dram_tensor("maskCT", [K, L], f32, kind="ExternalInput")
    ident = nc.dram_tensor("ident", [128, 128], bf16, kind="ExternalInput")
    ident82 = nc.dram_tensor("ident82", [128, 256], fp8, kind="ExternalInput")
    out_mnum = nc.dram_tensor("out_mnum", [EM, HA], bf16, kind="ExternalOutput")
    out_v = nc.dram_tensor("out_v", [4 * K, HA], bf16, kind="ExternalOutput")

    with tile.TileContext(nc) as tc:
        with (
            tc.tile_pool(name="const", bufs=1) as constp,
            tc.tile_pool(name="gat", bufs=3) as gatp,
            tc.tile_pool(name="acc", bufs=1) as accp,
            tc.tile_pool(name="stage", bufs=1) as stagep,
            tc.tile_pool(name="pshold", bufs=1, space="PSUM") as pshold,
            tc.tile_pool(name="psrot", bufs=2, space="PSUM") as psrot,
        ):
            # ---- indices first: gathers are the critical path ----
            idxa_s = constp.tile([128, nchunk], i32, tag="idxa", name="idxa")
            nc.sync.dma_start(out=idxa_s[:], in_=idx_att[:])

            g5s = []
            for q in range(nchunk):
                g5 = gatp.tile([128, HPG * L], fp8, tag="g5", name="g5")
                nc.gpsimd.indirect_dma_start(
                    out=g5[:], out_offset=None, in_=att_r[:],
                    in_offset=bass.IndirectOffsetOnAxis(ap=idxa_s[:, q:q + 1], axis=0))
                g5s.append(g5)

            # ---- remaining consts; seqb rides the gpsimd queue BEHIND the
            #      gathers so its packets don't race the critical gather ----
            maskS_s = constp.tile([128, nchunk, K], bf16, tag="maskS", name="maskS")
            maskCT_s = constp.tile([K, L], f32, tag="maskCT", name="maskCT")
            ident_s = constp.tile([128, 128], bf16, tag="ident", name="ident")
            ident82_s = constp.tile([128, 256], fp8, tag="ident82", name="ident82")
            seq_s = constp.tile([128, CT, HA], bf16, tag="seqs", name="seqs")
            nc.sync.dma_start(out=maskS_s[:], in_=maskS[:].rearrange("p (q k) -> p q k", k=K))
            nc.scalar.dma_start(out=maskCT_s[:], in_=maskCT[:])
            nc.sync.dma_start(out=ident_s[:], in_=ident[:])
            nc.sync.dma_start(out=ident82_s[:], in_=ident82[:])
            nc.scalar.dma_start(out=seq_s[:], in_=seqb[:].rearrange("(c p) f -> p c f", p=128))

            # ---- head-sum on PE (identity-weight accumulation over 6 heads) ----
            gs_s = accp.tile([128, nchunk, L], bf16, tag="gs", name="gs")
            gtm_s = accp.tile([128, CT, 128], bf16, tag="gtm", name="gtm")
            wv_s = accp.tile([128, CT, K], bf16, tag="wv", name="wv")
            put0 = pshold.tile([K, 512], f32, tag="put0", name="put0")
            put1 = pshold.tile([K, 512], f32, tag="put1", name="put1")
            for q in range(nchunk):
                g5r = g5s[q][:].rearrange("p (h c) -> p h c", h=HPG)
                for half in range(2):
                    ph = psrot.tile([128, 512], f32, tag="ph", name="ph")
                    for hp in range(HPG // 2):
                        nc.tensor.matmul(
                            ph[:], ident82_s[:].rearrange("p (two m) -> p two m", two=2),
                            g5r[:, ds(2 * hp, 2), ds(half * 512, 512)],
                            start=(hp == 0), stop=(hp == HPG // 2 - 1),
                            perf_mode=mybir.MatmulPerfMode.DoubleRow)
                    if half == 0:
                        nc.scalar.copy(out=gs_s[:, q, ds(half * 512, 512)], in_=ph[:])
                    else:
                        nc.vector.tensor_copy(gs_s[:, q, ds(half * 512, 512)], ph[:])
                # span-row reduction (transposed): uT[k,c] += maskS^T @ gs_q
                nc.tensor.matmul(put0[:], maskS_s[:, q, :], gs_s[:, q, 0:512],
                                 start=(q == 0), stop=(q == nchunk - 1))
                nc.tensor.matmul(put1[:], maskS_s[:, q, :], gs_s[:, q, 512:L],
                                 start=(q == 0), stop=(q == nchunk - 1))
                if q == 0:
                    # mention rows of S: PE-transpose of gathered chunk 0
                    for ct in range(CT):
                        pt = psrot.tile([128, 128], bf16, tag="pt", name="pt")
                        nc.tensor.transpose(pt[:], gs_s[:, 0, ts(ct, 128)], ident_s[:])
                        nc.vector.tensor_copy(gtm_s[:, ct, :], pt[:])

            # ---- mnum: mention-context numerators (runs early, off gtm) ----
            pm0 = pshold.tile([EM, 512], f32, tag="pm0", name="pm0")
            pm1 = pshold.tile([EM, HA - 512], f32, tag="pm1", name="pm1")
            for ct in range(CT):
                st, sp = (ct == 0), (ct == CT - 1)
                nc.tensor.matmul(pm0[:], gtm_s[:, ct, :], seq_s[:, ct, 0:512], start=st, stop=sp)
                nc.tensor.matmul(pm1[:], gtm_s[:, ct, :], seq_s[:, ct, 512:HA], start=st, stop=sp)
            mnum_s = stagep.tile([EM, HA], bf16, tag="mnum", name="mnum")
            nc.scalar.copy(out=mnum_s[:, 0:512], in_=pm0[:])
            nc.scalar.copy(out=mnum_s[:, 512:HA], in_=pm1[:])
            nc.sync.dma_start(out=out_mnum[:], in_=mnum_s[:])

            # ---- wvT = uT * column-mask, transpose back, col-tiled v matmuls ----
            wvt_s = accp.tile([K, L], bf16, tag="wvt", name="wvt")
            nc.vector.tensor_mul(wvt_s[:, 0:512], put0[:], maskCT_s[:, 0:512])
            nc.vector.tensor_mul(wvt_s[:, 512:L], put1[:], maskCT_s[:, 512:L])
            for ct in range(CT):
                ptk = psrot.tile([128, 128], bf16, tag="pt", name="ptk")
                nc.tensor.transpose(ptk[:, 0:K], wvt_s[:, ts(ct, 128)], ident_s[0:K, 0:K])
                nc.vector.tensor_copy(wv_s[:, ct, :], ptk[:, 0:K])
            pv0 = psrot.tile([128, 512], f32, tag="ph", name="pv0")
            pv1 = psrot.tile([128, 512], f32, tag="ph", name="pv1")
            v_s = stagep.tile([128, HA], bf16, tag="v", name="v")
            for ct in range(CT):
                grp = ct // 2
                st, sp = (ct % 2 == 0), (ct % 2 == 1)
                nc.tensor.matmul(pv0[ds(32 * grp, K), :], wv_s[:, ct, :],
                                 seq_s[:, ct, 0:512], start=st, stop=sp,
                                 tile_position=(0, 32 * grp))
                nc.tensor.matmul(pv1[ds(32 * grp, K), 0:HA - 512], wv_s[:, ct, :],
                                 seq_s[:, ct, 512:HA], start=st, stop=sp,
                                 tile_position=(0, 32 * grp))
            for grp in range(4):
                if grp % 2 == 0:
                    nc.scalar.copy(out=v_s[ds(32 * grp, K), 0:512],
                                   in_=pv0[ds(32 * grp, K), :])
                    nc.scalar.copy(out=v_s[ds(32 * grp, K), 512:HA],
                                   in_=pv1[ds(32 * grp, K), 0:HA - 512])
                else:
                    nc.vector.tensor_copy(v_s[ds(32 * grp, K), 0:512],
                                          pv0[ds(32 * grp, K), :])
                    nc.vector.tensor_copy(v_s[ds(32 * grp, K), 512:HA],
                                          pv1[ds(32 * grp, K), 0:HA - 512])
            for grp in range(4):
                nc.sync.dma_start(out=out_v[ts(grp, K), :], in_=v_s[ds(32 * grp, K), :])

    nc.compile()
    return nc


_NC_CACHE = {}


def _get_nc(nchunk=NCHUNK_DEF):
    if nchunk not in _NC_CACHE:
        _NC_CACHE[nchunk] = _build_nc(nchunk)
    return _NC_CACHE[nchunk]


def _per_core_inputs(sequence_output, attention, mention_pos, link_start, link_len):
    """Returns (in_maps for 8 cores, per-doc span lengths, nchunk)."""
    import ml_dtypes
    seq = np.ascontiguousarray(np.asarray(sequence_output, dtype=np.float32))
    att = np.asarray(attention)
    mpos = np.asarray(mention_pos).astype(np.int64)
    lstart = np.asarray(link_start).astype(np.int64)
    llen = np.asarray(link_len).astype(np.int64)

    docs = []
    max_slots = 0
    for b in range(B):
        pos = (mpos[b] + OFFSET).reshape(EM)
        s = lstart[b] + OFFSET
        e = lstart[b] + llen[b] + 1 + OFFSET
        row2slot = {}
        slots = list(pos)
        for j, r in enumerate(pos):
            row2slot.setdefault(int(r), j)
        for si, ei in zip(s, e):
            for r in range(int(si), int(ei)):
                if r not in row2slot:
                    row2slot[r] = len(slots)
                    slots.append(r)
        docs.append((pos, s, e, row2slot, slots))
        max_slots = max(max_slots, len(slots))
    nchunk = max(NCHUNK_DEF, -(-max_slots // 128))

    ident = np.eye(128, dtype=ml_dtypes.bfloat16)
    i8 = np.eye(128, dtype=ml_dtypes.float8_e4m3fn)
    ident82 = np.ascontiguousarray(np.concatenate([i8, i8], axis=1))
    in_maps = []
    lengths = []
    membs = []
    for b in range(B):
        pos, s, e, row2slot, slots = docs[b]
        n_slots = nchunk * 128
        sl = np.zeros(n_slots, np.int32)
        sl[:len(slots)] = slots
        idx_att = np.ascontiguousarray(sl.reshape(nchunk, 128).T)
        mS = np.zeros((n_slots, K), np.float32)
        mC = np.zeros((L, K), np.float32)
        for k, (si, ei) in enumerate(zip(s, e)):
            mC[int(si):int(ei), k] = 1.0
            for r in range(int(si), int(ei)):
                mS[row2slot[r], k] = 1.0
        maskS = np.ascontiguousarray(
            mS.reshape(nchunk, 128, K).transpose(1, 0, 2).reshape(128, nchunk * K)
        ).astype(ml_dtypes.bfloat16)
        maskCT = np.ascontiguousarray(mC.T)
        seqb = np.concatenate(
            [seq[b], np.ones((L, 1), np.float32), np.zeros((L, HA - H - 1), np.float32)],
            axis=1).astype(ml_dtypes.bfloat16)
        lengths.append((e - s).astype(np.float32))
        membs.append(seq[b][pos])  # mention embeddings: pure input row-gather
        for g in range(2):
            att_r = np.ascontiguousarray(
                att[b, g * HPG:(g + 1) * HPG].astype(ml_dtypes.float8_e4m3fn)
                .transpose(1, 0, 2).reshape(L, HPG * L))
            in_maps.append({
                "att_r": att_r, "seqb": seqb, "idx_att": idx_att,
                "maskS": maskS, "maskCT": maskCT, "ident": ident, "ident82": ident82,
            })
    return in_maps, lengths, membs, nchunk


def _combine(outs, lengths, membs, type_table):
    ttab = np.asarray(type_table, dtype=np.float32)
    type_ids = np.concatenate(
        [np.zeros(E, np.int64), np.ones(EM, np.int64), np.full(K, 2, np.int64)])
    nodes_type = ttab[type_ids]  # [E+EM+K, TYPE_DIM]

    out = np.zeros((B, E + EM + K + E + EM, H + TYPE_DIM), np.float32)
    for b in range(B):
        o0, o1 = outs[2 * b], outs[2 * b + 1]
        v4 = np.asarray(o0["out_v"], np.float32) + np.asarray(o1["out_v"], np.float32)
        v = v4.reshape(4, K, HA).sum(axis=0)
        mnum = (np.asarray(o0["out_mnum"], np.float32)
                + np.asarray(o1["out_mnum"], np.float32))
        memb = membs[b]
        length = lengths[b]

        link_rep = v[:, :H] / (NH * length[:, None])
        m_ctx = mnum[:, :H] / (mnum[:, H:H + 1] + NH * 1e-5)
        enum = mnum.reshape(E, MPE, HA).sum(axis=1)
        e_ctx = enum[:, :H] / (enum[:, H:H + 1] + NH * MPE * 1e-5)

        mg = memb.reshape(E, MPE, H)
        mmax = mg.max(axis=1)
        eemb = np.log(np.exp(mg - mmax[:, None, :]).sum(axis=1)) + mmax

        nodes_raw = np.concatenate([eemb, memb, link_rep], axis=0)      # [176,H]
        nodes = np.concatenate([nodes_raw, nodes_type], axis=1)         # [176,H+20]
        ctx = np.concatenate([e_ctx, m_ctx], axis=0)                    # [160,H]
        ctx = np.concatenate([ctx, np.zeros((E + EM, TYPE_DIM), np.float32)], axis=1)
        out[b] = np.concatenate([nodes, ctx], axis=0)
    return out


def kernel(**inputs):
    from concourse.bass_utils import run_bass_kernel_spmd

    in_maps, lengths, membs, nchunk = _per_core_inputs(
        inputs["sequence_output"], inputs["attention"],
        inputs["mention_pos"], inputs["link_start"], inputs["link_len"])
    nc = _get_nc(nchunk)
    res = run_bass_kernel_spmd(nc, in_maps, core_ids=list(range(8)))
    return _combine(res.results, lengths, membs, inputs["type_table"])


# revision 46
# speedup vs baseline: 1.5240x; 1.2544x over previous
"""Trainium2 Bass kernel for nn_DocREModel (doc-level relation extraction graph pooling).

Strategy (8 NeuronCores): each doc b (B=4) is split across 2 cores by attention
COLUMNS (each core holds all 12 heads x 512 columns).  Key observation: the
model only ever reads attention rows at mention positions (<=128 distinct) and
inside link spans, i.e. ~35% of the [1024,1024] matrix.  Each core therefore
device-GATHERS just those rows (SWDGE indirect DMA with a runtime per-partition
index tile), head-sums them on the PE (fp8 DoubleRow identity matmuls, two
heads per pass), and runs small PE matmuls:

  - att_r HBM layout [row, head*512+col] (fp8): gathering one row pulls all 12
    heads' half-columns of that row contiguously (6 KB/descriptor).
  - gather slot j<128 == mention j (EM=128 exactly), so the "onehot" gather
    matrix is the identity: mention rows of S come from a PE transpose of
    gathered chunk 0.  Span rows occupy slots >=128; a host-built slot mask
    [slots,16] reduces them to per-span row-sums uT[16,512] via PE matmul.
  - mnum = S_mention^T @ [seq|1]  (context numerators + row-sums; partial over
    this core's 512 columns - host sums the pair)
  - v    = (uT*maskCT)^T-transposed-back @ [seq|1], col-group-tiled 4x on the
    PE (16-row outputs packed at partition offsets 0/32/64/96; host sums).
The host gathers mention embeddings (pure input row-gather) and applies the
tiny normalizations (head-count / span-length / row-sum divides, entity
pooling, 4-way logsumexp) while unsharding.
"""

import os
import sys

for _p in ("/opt/trn_rl_repo", "/root/.axon_site/_ro/trn_rl_repo"):
    if os.path.isdir(_p) and _p not in sys.path:
        sys.path.insert(0, _p)

import numpy as np

B, L, H, NH = 4, 1024, 768, 12
E, MPE, K = 32, 4, 16
EM = E * MPE              # 128 mentions per doc == gather chunk 0
TYPE_DIM = 20
OFFSET = 1
CW = L // 2               # columns per core (2 cores per doc)
CTH = CW // 128           # 4 column chunks per core
HA = H + 4                # seq | ones | 3 zero-pad -> 772 (row-sum in col 768)
NCHUNK_DEF = 3            # gather slots = 128*NCHUNK (>= 128 mentions + span rows)


def _build_nc(nchunk=NCHUNK_DEF, debug=False):
    import concourse.bass as bass
    import concourse.mybir as mybir
    import concourse.tile as tile
    from concourse import bacc

    f32 = mybir.dt.float32
    bf16 = mybir.dt.bfloat16
    fp8 = mybir.dt.float8e4  # e4m3
    i32 = mybir.dt.int32
    ts, ds = bass.ts, bass.ds

    nc = bacc.Bacc("TRN2", target_bir_lowering=False, debug=debug)

    att_r = nc.dram_tensor("att_r", [L, NH * CW], fp8, kind="ExternalInput")
    seqb = nc.dram_tensor("seqb", [CW, HA], bf16, kind="ExternalInput")
    idx_att = nc.dram_tensor("idx_att", [128, nchunk], i32, kind="ExternalInput")
    maskS = nc.dram_tensor("maskS", [128, nchunk * K], bf16, kind="ExternalInput")
    maskCT = nc.dram_tensor("maskCT", [K, CW], f32, kind="ExternalInput")
    ident = nc.dram_tensor("ident", [128, 128], bf16, kind="ExternalInput")
    ident82 = nc.dram_tensor("ident82", [128, 256], fp8, kind="ExternalInput")
    out_mnum = nc.dram_tensor("out_mnum", [EM, HA], bf16, kind="ExternalOutput")
    out_v = nc.dram_tensor("out_v", [4 * K, HA], bf16, kind="ExternalOutput")

    with tile.TileContext(nc) as tc:
        with (
            tc.tile_pool(name="const", bufs=1) as constp,
            tc.tile_pool(name="gat", bufs=3) as gatp,
            tc.tile_pool(name="acc", bufs=1) as accp,
            tc.tile_pool(name="stage", bufs=1) as stagep,
            tc.tile_pool(name="pshold", bufs=1, space="PSUM") as pshold,
            tc.tile_pool(name="psrot", bufs=2, space="PSUM") as psrot,
        ):
            # ---- indices first: gathers are the critical path ----
            idxa_s = constp.tile([128, nchunk], i32, tag="idxa", name="idxa")
            nc.sync.dma_start(out=idxa_s[:], in_=idx_att[:])

            g5s = []
            for q in range(nchunk):
                g5 = gatp.tile([128, NH * CW], fp8, tag="g5", name="g5")
                nc.gpsimd.indirect_dma_start(
                    out=g5[:], out_offset=None, in_=att_r[:],
                    in_offset=bass.IndirectOffsetOnAxis(ap=idxa_s[:, q:q + 1], axis=0))
                g5s.append(g5)

            # ---- remaining consts ----
            maskS_s = constp.tile([128, nchunk, K], bf16, tag="maskS", name="maskS")
            maskCT_s = constp.tile([K, CW], f32, tag="maskCT", name="maskCT")
            ident_s = constp.tile([128, 128], bf16, tag="ident", name="ident")
            ident82_s = constp.tile([128, 256], fp8, tag="ident82", name="ident82")
            seq_s = constp.tile([128, CTH, HA], bf16, tag="seqs", name="seqs")
            nc.sync.dma_start(out=maskS_s[:], in_=maskS[:].rearrange("p (q k) -> p q k", k=K))
            nc.scalar.dma_start(out=maskCT_s[:], in_=maskCT[:])
            nc.sync.dma_start(out=ident_s[:], in_=ident[:])
            nc.sync.dma_start(out=ident82_s[:], in_=ident82[:])
            nc.scalar.dma_start(out=seq_s[:], in_=seqb[:].rearrange("(c p) f -> p c f", p=128))

            # ---- head-sum on PE: fp8 DoubleRow identity matmuls, 2 heads/pass ----
            gs_s = accp.tile([128, nchunk, CW], bf16, tag="gs", name="gs")
            gtm_s = accp.tile([128, CTH, 128], bf16, tag="gtm", name="gtm")
            wv_s = accp.tile([128, CTH, K], bf16, tag="wv", name="wv")
            put = pshold.tile([K, CW], f32, tag="put", name="put")
            for q in range(nchunk):
                g5r = g5s[q][:].rearrange("p (h c) -> p h c", h=NH)
                ph = psrot.tile([128, 512], f32, tag="ph", name="ph")
                for hp in range(NH // 2):
                    nc.tensor.matmul(
                        ph[:], ident82_s[:].rearrange("p (two m) -> p two m", two=2),
                        g5r[:, ds(2 * hp, 2), :],
                        start=(hp == 0), stop=(hp == NH // 2 - 1),
                        perf_mode=mybir.MatmulPerfMode.DoubleRow)
                nc.scalar.copy(out=gs_s[:, q, :], in_=ph[:])
                # span-row reduction (transposed): uT[k,c] += maskS^T @ gs_q
                nc.tensor.matmul(put[:], maskS_s[:, q, :], gs_s[:, q, :],
                                 start=(q == 0), stop=(q == nchunk - 1))
                if q == 0:
                    # mention rows of S: PE-transpose of gathered chunk 0
                    for ct in range(CTH):
                        pt = psrot.tile([128, 128], bf16, tag="pt", name="pt")
                        nc.tensor.transpose(pt[:], gs_s[:, 0, ts(ct, 128)], ident_s[:])
                        nc.vector.tensor_copy(gtm_s[:, ct, :], pt[:])

            # ---- mnum: mention-context numerators (partial over this c-half) ----
            pm0 = pshold.tile([EM, 512], f32, tag="pm0", name="pm0")
            pm1 = pshold.tile([EM, HA - 512], f32, tag="pm1", name="pm1")
            for ct in range(CTH):
                st, sp = (ct == 0), (ct == CTH - 1)
                nc.tensor.matmul(pm0[:], gtm_s[:, ct, :], seq_s[:, ct, 0:512], start=st, stop=sp)
                nc.tensor.matmul(pm1[:], gtm_s[:, ct, :], seq_s[:, ct, 512:HA], start=st, stop=sp)
            mnum_s = stagep.tile([EM, HA], bf16, tag="mnum", name="mnum")
            nc.scalar.copy(out=mnum_s[:, 0:512], in_=pm0[:])
            nc.scalar.copy(out=mnum_s[:, 512:HA], in_=pm1[:])
            nc.sync.dma_start(out=out_mnum[:], in_=mnum_s[:])

            # ---- wvT = uT * column-mask, transpose back, col-tiled v matmuls ----
            wvt_s = accp.tile([K, CW], bf16, tag="wvt", name="wvt")
            nc.vector.tensor_mul(wvt_s[:], put[:], maskCT_s[:])
            for ct in range(CTH):
                ptk = psrot.tile([128, 128], bf16, tag="pt", name="ptk")
                nc.tensor.transpose(ptk[:, 0:K], wvt_s[:, ts(ct, 128)], ident_s[0:K, 0:K])
                nc.vector.tensor_copy(wv_s[:, ct, :], ptk[:, 0:K])
            pv0 = psrot.tile([128, 512], f32, tag="ph", name="pv0")
            pv1 = psrot.tile([128, 512], f32, tag="ph", name="pv1")
            v_s = stagep.tile([128, HA], bf16, tag="v", name="v")
            for ct in range(CTH):
                nc.tensor.matmul(pv0[ds(32 * ct, K), :], wv_s[:, ct, :],
                                 seq_s[:, ct, 0:512], start=True, stop=True,
                                 tile_position=(0, 32 * ct))
                nc.tensor.matmul(pv1[ds(32 * ct, K), 0:HA - 512], wv_s[:, ct, :],
                                 seq_s[:, ct, 512:HA], start=True, stop=True,
                                 tile_position=(0, 32 * ct))
            for grp in range(4):
                if grp % 2 == 0:
                    nc.scalar.copy(out=v_s[ds(32 * grp, K), 0:512],
                                   in_=pv0[ds(32 * grp, K), :])
                    nc.scalar.copy(out=v_s[ds(32 * grp, K), 512:HA],
                                   in_=pv1[ds(32 * grp, K), 0:HA - 512])
                else:
                    nc.vector.tensor_copy(v_s[ds(32 * grp, K), 0:512],
                                          pv0[ds(32 * grp, K), :])
                    nc.vector.tensor_copy(v_s[ds(32 * grp, K), 512:HA],
                                          pv1[ds(32 * grp, K), 0:HA - 512])
            for grp in range(4):
                nc.sync.dma_start(out=out_v[ts(grp, K), :], in_=v_s[ds(32 * grp, K), :])

    nc.compile()
    return nc


_NC_CACHE = {}


def _get_nc(nchunk=NCHUNK_DEF):
    if nchunk not in _NC_CACHE:
        _NC_CACHE[nchunk] = _build_nc(nchunk)
    return _NC_CACHE[nchunk]


def _per_core_inputs(sequence_output, attention, mention_pos, link_start, link_len):
    """Returns (in_maps for 8 cores, per-doc span lengths, per-doc membs, nchunk)."""
    import ml_dtypes
    seq = np.ascontiguousarray(np.asarray(sequence_output, dtype=np.float32))
    att = np.asarray(attention)
    mpos = np.asarray(mention_pos).astype(np.int64)
    lstart = np.asarray(link_start).astype(np.int64)
    llen = np.asarray(link_len).astype(np.int64)

    docs = []
    max_slots = 0
    for b in range(B):
        pos = (mpos[b] + OFFSET).reshape(EM)
        s = lstart[b] + OFFSET
        e = lstart[b] + llen[b] + 1 + OFFSET
        row2slot = {}
        slots = list(pos)
        for j, r in enumerate(pos):
            row2slot.setdefault(int(r), j)
        for si, ei in zip(s, e):
            for r in range(int(si), int(ei)):
                if r not in row2slot:
                    row2slot[r] = len(slots)
                    slots.append(r)
        docs.append((pos, s, e, row2slot, slots))
        max_slots = max(max_slots, len(slots))
    nchunk = max(NCHUNK_DEF, -(-max_slots // 128))

    ident = np.eye(128, dtype=ml_dtypes.bfloat16)
    i8 = np.eye(128, dtype=ml_dtypes.float8_e4m3fn)
    ident82 = np.ascontiguousarray(np.concatenate([i8, i8], axis=1))
    in_maps = []
    lengths = []
    membs = []
    for b in range(B):
        pos, s, e, row2slot, slots = docs[b]
        n_slots = nchunk * 128
        sl = np.zeros(n_slots, np.int32)
        sl[:len(slots)] = slots
        idx_att = np.ascontiguousarray(sl.reshape(nchunk, 128).T)
        mS = np.zeros((n_slots, K), np.float32)
        mC = np.zeros((L, K), np.float32)
        for k, (si, ei) in enumerate(zip(s, e)):
            mC[int(si):int(ei), k] = 1.0
            for r in range(int(si), int(ei)):
                mS[row2slot[r], k] = 1.0
        maskS = np.ascontiguousarray(
            mS.reshape(nchunk, 128, K).transpose(1, 0, 2).reshape(128, nchunk * K)
        ).astype(ml_dtypes.bfloat16)
        seqb_full = np.concatenate(
            [seq[b], np.ones((L, 1), np.float32), np.zeros((L, HA - H - 1), np.float32)],
            axis=1).astype(ml_dtypes.bfloat16)
        lengths.append((e - s).astype(np.float32))
        membs.append(seq[b][pos])  # mention embeddings: pure input row-gather
        att8 = att[b].astype(ml_dtypes.float8_e4m3fn)  # [12, L, L]
        for g in range(2):
            cols = slice(g * CW, (g + 1) * CW)
            att_r = np.ascontiguousarray(
                att8[:, :, cols].transpose(1, 0, 2).reshape(L, NH * CW))
            in_maps.append({
                "att_r": att_r,
                "seqb": np.ascontiguousarray(seqb_full[cols]),
                "idx_att": idx_att, "maskS": maskS,
                "maskCT": np.ascontiguousarray(mC.T[:, cols]),
                "ident": ident, "ident82": ident82,
            })
    return in_maps, lengths, membs, nchunk


def _combine(outs, lengths, membs, type_table):
    ttab = np.asarray(type_table, dtype=np.float32)
    type_ids = np.concatenate(
        [np.zeros(E, np.int64), np.ones(EM, np.int64), np.full(K, 2, np.int64)])
    nodes_type = ttab[type_ids]  # [E+EM+K, TYPE_DIM]

    out = np.zeros((B, E + EM + K + E + EM, H + TYPE_DIM), np.float32)
    for b in range(B):
        o0, o1 = outs[2 * b], outs[2 * b + 1]
        v4 = np.asarray(o0["out_v"], np.float32) + np.asarray(o1["out_v"], np.float32)
        v = v4.reshape(4, K, HA).sum(axis=0)
        mnum = (np.asarray(o0["out_mnum"], np.float32)
                + np.asarray(o1["out_mnum"], np.float32))
        memb = membs[b]
        length = lengths[b]

        link_rep = v[:, :H] / (NH * length[:, None])
        m_ctx = mnum[:, :H] / (mnum[:, H:H + 1] + NH * 1e-5)
        enum = mnum.reshape(E, MPE, HA).sum(axis=1)
        e_ctx = enum[:, :H] / (enum[:, H:H + 1] + NH * MPE * 1e-5)

        mg = memb.reshape(E, MPE, H)
        mmax = mg.max(axis=1)
        eemb = np.log(np.exp(mg - mmax[:, None, :]).sum(axis=1)) + mmax

        nodes_raw = np.concatenate([eemb, memb, link_rep], axis=0)      # [176,H]
        nodes = np.concatenate([nodes_raw, nodes_type], axis=1)         # [176,H+20]
        ctx = np.concatenate([e_ctx, m_ctx], axis=0)                    # [160,H]
        ctx = np.concatenate([ctx, np.zeros((E + EM, TYPE_DIM), np.float32)], axis=1)
        out[b] = np.concatenate([nodes, ctx], axis=0)
    return out


def kernel(**inputs):
    from concourse.bass_utils import run_bass_kernel_spmd

    in_maps, lengths, membs, nchunk = _per_core_inputs(
        inputs["sequence_output"], inputs["attention"],
        inputs["mention_pos"], inputs["link_start"], inputs["link_len"])
    nc = _get_nc(nchunk)
    res = run_bass_kernel_spmd(nc, in_maps, core_ids=list(range(8)))
    return _combine(res.results, lengths, membs, inputs["type_table"])
